# revision 1
# baseline (speedup 1.0000x reference)
"""Trainium2 Bass kernel for gated dense attention with dim=0 softmax.

Computation (reference):
    h = x @ W1 + b1
    q,k,v = h @ W{q,k,v} + b{q,k,v}
    w = (q @ k.T) / sqrt(256)
    attn = softmax(w, axis=0)          # normalizes over ROWS per column
    h2 = a*h + (1-a)*(attn @ v)
    out = h2 @ W2 + b2

Sharding: rows (N=8192) split across 8 cores (1024 each). Each core owns
S^T[j, i_local] for ALL columns j; the softmax denominator (a column sum
over all rows) is completed with AllReduce over cores, split in quarters
and pipelined behind compute. k^T and v are AllGathered in fp8. The two
N x N matmuls (S = q k^T and attn @ v) run in fp8e4 with DoubleRow perf
mode; exp + per-column sum are fused on ScalarE (accum_out), which is
the pacing engine (~1 elem/cycle/lane).

Precision plan: the attention path (q,k,v,S,P) tolerates fp8/bf16 since
its errors average over 8192 terms; it is fed from a bf16 h. The
residual path a*h and the output projection stay fp32 (a separate fp32
h matmul runs overlapped with the S phase). Magnitude bookkeeping for
fp8: v' = v * 8192*(1-a)/s_j keeps v' in fp8 normal range; 1/8192 is
folded into W2 on the host and the gate a is scaled by 8192 to match.
"""

import numpy as np
import ml_dtypes

BF16 = ml_dtypes.bfloat16
FP8 = ml_dtypes.float8_e4m3fn

N, D, H, C = 8192, 512, 256, 256
NCORES = 8
NL = N // NCORES          # 1024 rows per core
JT = N // 128             # 64 j-tiles (columns of the attention matrix)
NPAIR = JT // 2           # 32 DoubleRow j-pairs for the PV contraction
# softmax-denominator AllReduce groups: uneven so the LAST group (whose
# AllReduce latency is exposed in the tail) is small
GROUPS = [16, 16, 24, 8]
GSTART = [0, 16, 32, 56]
NQ = len(GROUPS)
PV_LAG = 40               # S runs this many j-tiles ahead before PV interleaves
VSCALE = 8192.0           # fp8 range shim: v' scaled up, W2 scaled down

K_SEC = 2 * 128 * NL      # fp8 elements of k^T per core in the gather buffer
V_SEC = NL * C            # fp8 elements of v per core

_CACHED = {}


def _build():
    import concourse.mybir as mybir
    from concourse import bacc
    from concourse.tile import TileContext

    dt = mybir.dt
    AF = mybir.ActivationFunctionType
    DR = mybir.MatmulPerfMode.DoubleRow
    f32, bf, f8 = dt.float32, dt.bfloat16, dt.float8e4
    RG = [list(range(NCORES))]

    nc = bacc.Bacc(None, target_bir_lowering=False, num_devices=NCORES)

    # ---------------- I/O (per core) ----------------
    xT = nc.declare_dram_parameter("xT", [128, 4, NL], f32, isOutput=False)
    xTb = nc.declare_dram_parameter("xTb", [128, 4, NL], bf, isOutput=False)
    w1 = nc.declare_dram_parameter("w1", [128, 4, H], f32, isOutput=False)
    w1b = nc.declare_dram_parameter("w1b", [128, 4, H], bf, isOutput=False)
    wq = nc.declare_dram_parameter("wq", [128, 2, H], bf, isOutput=False)
    wk = nc.declare_dram_parameter("wk", [128, 2, H], bf, isOutput=False)
    wv = nc.declare_dram_parameter("wv", [128, 2, H], bf, isOutput=False)
    w2 = nc.declare_dram_parameter("w2", [128, 2, C], f32, isOutput=False)
    b1c = nc.declare_dram_parameter("b1c", [128, 2], f32, isOutput=False)
    bqc = nc.declare_dram_parameter("bqc", [128, 2], f32, isOutput=False)
    bkc = nc.declare_dram_parameter("bkc", [128, 2], f32, isOutput=False)
    bvb = nc.declare_dram_parameter("bvb", [128, C], f32, isOutput=False)
    b2b = nc.declare_dram_parameter("b2b", [128, C], f32, isOutput=False)
    acol = nc.declare_dram_parameter("acol", [128, 1], f32, isOutput=False)
    omac = nc.declare_dram_parameter("omac", [128, 1], f32, isOutput=False)
    y = nc.declare_dram_parameter("y", [NL, C], f32, isOutput=True)

    ccin_k = nc.dram_tensor("ccin_k", [K_SEC], f8)
    ccout_k = nc.dram_tensor("ccout_k", [NCORES * K_SEC], f8, addr_space="Shared")
    ccin_v = nc.dram_tensor("ccin_v", [V_SEC], f8)
    ccout_v = nc.dram_tensor("ccout_v", [NCORES * V_SEC], f8, addr_space="Shared")
    sin = [nc.dram_tensor(f"sin{q}", [128 * GROUPS[q]], f32) for q in range(NQ)]
    sout = [
        nc.dram_tensor(f"sout{q}", [128 * GROUPS[q]], f32, addr_space="Shared")
        for q in range(NQ)
    ]

    def cc_k(g):
        # k^T chunk of rank g: [128, 2, 1024] view (d_lo, d_hi, j_local)
        off = g * K_SEC
        return ccout_k[off : off + K_SEC].rearrange(
            "(e p f) -> p e f", e=2, p=128
        )

    def cc_v(jt):
        g, t = jt // 8, jt % 8
        off = g * V_SEC + t * (128 * C)
        return ccout_v[off : off + 128 * C].rearrange("(p f) -> p f", p=128)

    with TileContext(nc) as tc:
        with (
            tc.tile_pool(name="cst", bufs=1) as cst,
            tc.tile_pool(name="big", bufs=1) as big,
            tc.tile_pool(name="ptp", bufs=NPAIR) as ptp,
            tc.tile_pool(name="strm", bufs=1) as strm,
            tc.tile_pool(name="psum", bufs=1, space="PSUM") as psum,
        ):
            # ---- constants (bf16 weights for the attention path first) ----
            w1bt = cst.tile([128, 4, H], bf, tag="w1bt", name="w1bt")
            nc.sync.dma_start(out=w1bt, in_=w1b[:])
            wkt = cst.tile([128, 2, H], bf, tag="wkt", name="wkt")
            nc.sync.dma_start(out=wkt, in_=wk[:])
            wqt = cst.tile([128, 2, H], bf, tag="wqt", name="wqt")
            nc.sync.dma_start(out=wqt, in_=wq[:])
            wvt = cst.tile([128, 2, H], bf, tag="wvt", name="wvt")
            nc.sync.dma_start(out=wvt, in_=wv[:])
            b1t = cst.tile([128, 2], f32, tag="b1t", name="b1t")
            nc.sync.dma_start(out=b1t, in_=b1c[:])
            bkt = cst.tile([128, 2], f32, tag="bkt", name="bkt")
            nc.sync.dma_start(out=bkt, in_=bkc[:])
            # dummy exp: forces the ACT table load during phase A instead of
            # at the first real exp on the critical path
            scr = cst.tile([128, 1], f32, tag="scr", name="scr")
            nc.scalar.activation(scr, b1t[:, 0:1], AF.Exp)
            bqt = cst.tile([128, 2], f32, tag="bqt", name="bqt")
            nc.sync.dma_start(out=bqt, in_=bqc[:])
            bvt = cst.tile([128, C], f32, tag="bvt", name="bvt")
            nc.sync.dma_start(out=bvt, in_=bvb[:])

            xtb = big.tile([128, 4, NL], bf, tag="xtb", name="xtb")
            for kt in range(4):
                nc.sync.dma_start(out=xtb[:, kt, :], in_=xTb[:, kt, :])

            # fp32/residual-path constants (less urgent)
            w1t = cst.tile([128, 4, H], f32, tag="w1t", name="w1t")
            nc.sync.dma_start(out=w1t, in_=w1[:])
            w2t = cst.tile([128, 2, C], f32, tag="w2t", name="w2t")
            nc.sync.dma_start(out=w2t, in_=w2[:])
            b2t = cst.tile([128, C], f32, tag="b2t", name="b2t")
            nc.sync.dma_start(out=b2t, in_=b2b[:])
            act = cst.tile([128, 1], f32, tag="act", name="act")
            nc.sync.dma_start(out=act, in_=acol[:])
            omt = cst.tile([128, 1], f32, tag="omt", name="omt")
            nc.sync.dma_start(out=omt, in_=omac[:])
            xts = big.tile([128, 4, NL], f32, tag="xts", name="xts")
            for kt in range(4):
                nc.sync.dma_start(out=xts[:, kt, :], in_=xT[:, kt, :])

            hts32 = big.tile([128, 2, NL], f32, tag="hts32", name="hts32")
            htsbf = big.tile([128, 2, NL], bf, tag="htsbf", name="htsbf")
            qts = big.tile([128, 2, NL], f8, tag="qts", name="qts")
            kts = big.tile([128, 2, NL], f8, tag="kts", name="kts")

            colsq = []
            for q in range(NQ):
                cs = big.tile([128, GROUPS[q]], f32, tag="colsq", bufs=NQ,
                              name=f"colsq{q}")
                colsq.append(cs)

            def grp(jt):
                for g in range(NQ):
                    if jt < GSTART[g] + GROUPS[g]:
                        return g, jt - GSTART[g]
                raise AssertionError(jt)

            # ============ phase A: bf16 h -> k -> AG-k -> v -> AG-v -> q ======
            # bf16 h feeds only the attention path; k-tile-outer so PE starts
            # on x chunk 0
            ph = [
                [
                    psum.tile([128, 512], f32, tag="A", bufs=4,
                              name=f"ph{m}{n}")
                    for n in range(2)
                ]
                for m in range(2)
            ]
            for kt in range(4):
                for m in range(2):
                    for n in range(2):
                        nc.tensor.matmul(
                            ph[m][n],
                            lhsT=w1bt[:, kt, m * 128 : (m + 1) * 128],
                            rhs=xtb[:, kt, n * 512 : (n + 1) * 512],
                            start=(kt == 0),
                            stop=(kt == 3),
                        )
            for m in range(2):
                for n in range(2):
                    # split across DVE/ScalarE: these gate the k matmuls on
                    # the AG-k critical path
                    if n == 0:
                        nc.vector.tensor_scalar_add(
                            htsbf[:, m, n * 512 : (n + 1) * 512],
                            ph[m][n],
                            b1t[:, m : m + 1],
                        )
                    else:
                        nc.scalar.activation(
                            htsbf[:, m, n * 512 : (n + 1) * 512],
                            ph[m][n],
                            AF.Identity,
                            bias=b1t[:, m : m + 1],
                        )

            # k^T (fp8) then DMA into the gather staging buffer
            for m in range(2):
                for n in range(2):
                    pk = psum.tile([128, 512], f32, tag="A", bufs=4, name="pk")
                    for kt in range(2):
                        nc.tensor.matmul(
                            pk,
                            lhsT=wkt[:, kt, m * 128 : (m + 1) * 128],
                            rhs=htsbf[:, kt, n * 512 : (n + 1) * 512],
                            start=(kt == 0),
                            stop=(kt == 1),
                        )
                    if n == 0:
                        nc.vector.tensor_scalar_add(
                            kts[:, m, n * 512 : (n + 1) * 512],
                            pk,
                            bkt[:, m : m + 1],
                        )
                    else:
                        # ScalarE takes half the k bias-adds: they drain in
                        # parallel with DVE on the AG-k critical path
                        nc.scalar.activation(
                            kts[:, m, n * 512 : (n + 1) * 512],
                            pk,
                            AF.Identity,
                            bias=bkt[:, m : m + 1],
                        )
            for m in range(2):
                off = m * (128 * NL)
                nc.sync.dma_start(
                    out=ccin_k[off : off + 128 * NL].rearrange(
                        "(p f) -> p f", p=128
                    ),
                    in_=kts[:, m, :],
                )
            # gather k first — it gates the S phase; v gather overlaps S
            nc.gpsimd.collective_compute(
                "AllGather",
                mybir.AluOpType.bypass,
                replica_groups=RG,
                ins=[ccin_k[:]],
                outs=[ccout_k[:]],
            )

            # q^T (fp8) — overlaps the AllGathers
            for m in range(2):
                for n in range(2):
                    pq = psum.tile([128, 512], f32, tag="A", bufs=4, name="pq")
                    for kt in range(2):
                        nc.tensor.matmul(
                            pq,
                            lhsT=wqt[:, kt, m * 128 : (m + 1) * 128],
                            rhs=htsbf[:, kt, n * 512 : (n + 1) * 512],
                            start=(kt == 0),
                            stop=(kt == 1),
                        )
                    nc.vector.tensor_scalar_add(
                        qts[:, m, n * 512 : (n + 1) * 512],
                        pq,
                        bqt[:, m : m + 1],
                    )

            # =================== phase S + PV (interleaved) ===================
            # PE filler units run in the slack of the ACT-paced S region
            # instead of delaying S(0): v production + its AllGather, and the
            # fp32 residual-h matmul.
            ph32 = [
                [
                    psum.tile([128, 512], f32, tag="A", bufs=4,
                              name=f"ph32{m}{n}")
                    for n in range(2)
                ]
                for m in range(2)
            ]

            def fill_v(it):
                pvv = psum.tile([128, C], f32, tag="A", bufs=4, name="pvv")
                for kt in range(2):
                    nc.tensor.matmul(
                        pvv,
                        lhsT=htsbf[:, kt, it * 128 : (it + 1) * 128],
                        rhs=wvt[:, kt, :],
                        start=(kt == 0),
                        stop=(kt == 1),
                    )
                vt = strm.tile([128, C], f8, tag="vt", bufs=3, name="vt")
                nc.vector.tensor_add(vt, pvv, bvt)
                off = it * (128 * C)
                nc.sync.dma_start(
                    out=ccin_v[off : off + 128 * C].rearrange(
                        "(p f) -> p f", p=128
                    ),
                    in_=vt,
                )

            def fill_agv(_):
                nc.gpsimd.collective_compute(
                    "AllGather",
                    mybir.AluOpType.bypass,
                    replica_groups=RG,
                    ins=[ccin_v[:]],
                    outs=[ccout_v[:]],
                )

            def fill_h32(u):
                kt, m, n = u
                nc.tensor.matmul(
                    ph32[m][n],
                    lhsT=w1t[:, kt, m * 128 : (m + 1) * 128],
                    rhs=xts[:, kt, n * 512 : (n + 1) * 512],
                    start=(kt == 0),
                    stop=(kt == 3),
                )

            def fill_h32fin(_):
                for m in range(2):
                    for n in range(2):
                        # hts32 = (h_psum + b1)*(a*8192) — residual ready-made
                        nc.vector.tensor_scalar(
                            hts32[:, m, n * 512 : (n + 1) * 512],
                            ph32[m][n],
                            b1t[:, m : m + 1],
                            act[:, 0:1],
                            op0=mybir.AluOpType.add,
                            op1=mybir.AluOpType.mult,
                        )

            fill = [(fill_v, it) for it in range(8)]
            fill.append((fill_agv, None))
            fill += [
                (fill_h32, (kt, m, n))
                for kt in range(4)
                for m in range(2)
                for n in range(2)
            ]
            fill.append((fill_h32fin, None))
            fill.reverse()

            PT2 = [None] * NPAIR     # fp8 [128, 2, 1024] exp(S/16) pair tiles
            ktg = {}
            invq = [None] * NQ

            def emit_S(jt):
                g, t = jt // 8, jt % 8
                if t == 0:
                    kg = strm.tile([128, 2, NL], f8, tag="ktg", bufs=2,
                                   name=f"ktg{g}")
                    nc.sync.dma_start(out=kg, in_=cc_k(g))
                    ktg[g] = kg
                kg = ktg[g]
                ps = psum.tile([128, NL], f32, tag="S", bufs=2, name="ps")
                for n in range(2):
                    nc.tensor.matmul(
                        ps[:, n * 512 : (n + 1) * 512],
                        lhsT=kg[:, :, t * 128 : (t + 1) * 128],
                        rhs=qts[:, :, n * 512 : (n + 1) * 512],
                        start=True,
                        stop=True,
                        perf_mode=DR,
                    )
                pair, e = jt // 2, jt % 2
                if PT2[pair] is None:
                    PT2[pair] = ptp.tile([128, 2, NL], f8, tag="pt",
                                         name=f"pt{pair}")
                qi, col = grp(jt)
                nc.scalar.activation(
                    PT2[pair][:, e, :],
                    ps,
                    AF.Exp,
                    scale=1.0 / 16.0,
                    accum_out=colsq[qi][:, col : col + 1],
                )

            def emit_stats(q):
                # stats staging lives on the gpsimd queue: its in-order stream
                # (AG, AR0, sq0, AR1, ...) matches these DMAs' dependencies,
                # while the SP queue keeps streaming ktg/v tiles unblocked
                nc.gpsimd.dma_start(
                    out=sin[q][:].rearrange("(p f) -> p f", p=128),
                    in_=colsq[q],
                )
                nc.gpsimd.collective_compute(
                    "AllReduce",
                    mybir.AluOpType.add,
                    replica_groups=RG,
                    ins=[sin[q][:]],
                    outs=[sout[q][:]],
                )
                sq = strm.tile([128, GROUPS[q]], f32, tag="sqs", bufs=2,
                               name="sq")
                nc.gpsimd.dma_start(
                    out=sq, in_=sout[q][:].rearrange("(p f) -> p f", p=128)
                )
                iv = big.tile([128, GROUPS[q]], f32, tag="invq", bufs=NQ,
                              name=f"invq{q}")
                nc.vector.reciprocal(iv, sq)
                # fold 8192*(1-a) (fp8 range shim + residual gate) into 1/s
                nc.vector.tensor_scalar_mul(iv, iv, omt[:, 0:1])
                invq[q] = iv

            def emit_PV(pair):
                vs = strm.tile([128, 2, C], f8, tag="vs", bufs=6, name="vs")
                for e in range(2):
                    jt = 2 * pair + e
                    vl = strm.tile([128, C], f8, tag="vl", bufs=12, name="vl")
                    nc.sync.dma_start(out=vl, in_=cc_v(jt))
                    qi, col = grp(jt)
                    nc.vector.tensor_scalar_mul(
                        vs[:, e, :], vl, invq[qi][:, col : col + 1]
                    )
                for c in range(2):
                    for n in range(2):
                        nc.tensor.matmul(
                            pv[c][n],
                            lhsT=vs[:, :, c * 128 : (c + 1) * 128],
                            rhs=PT2[pair][:, :, n * 512 : (n + 1) * 512],
                            start=(pair == 0),
                            stop=(pair == NPAIR - 1),
                            perf_mode=DR,
                        )

            # pv accumulators reuse the "A" slots (phase A psum all released)
            pv = [
                [
                    psum.tile([128, 512], f32, tag="A", bufs=4,
                              name=f"pv{c}{n}")
                    for n in range(2)
                ]
                for c in range(2)
            ]
            bounds = {GSTART[g] + GROUPS[g] - 1: g for g in range(NQ)}
            for jt in range(PV_LAG):
                emit_S(jt)
                if fill:
                    f, arg = fill.pop()
                    f(arg)
                if jt in bounds:
                    emit_stats(bounds[jt])
            npv = 0
            for jt in range(PV_LAG, JT):
                emit_S(jt)
                if jt in bounds:
                    emit_stats(bounds[jt])
                if (jt - PV_LAG) % 2 == 1:
                    emit_PV(npv)
                    npv += 1
            for pair in range(npv, NPAIR):
                emit_PV(pair)

            # =================== final: gated residual + out proj =============
            for n in range(2):
                for m in range(2):
                    nc.vector.tensor_add(
                        hts32[:, m, n * 512 : (n + 1) * 512],
                        hts32[:, m, n * 512 : (n + 1) * 512],
                        pv[m][n],
                    )
                for it in range(4 * n, 4 * n + 4):
                    py = psum.tile([128, C], f32, tag="S", bufs=2, name="py")
                    for kt in range(2):
                        nc.tensor.matmul(
                            py,
                            lhsT=hts32[:, kt, it * 128 : (it + 1) * 128],
                            rhs=w2t[:, kt, :],
                            start=(kt == 0),
                            stop=(kt == 1),
                        )
                    yt = strm.tile([128, C], f32, tag="yt", bufs=3,
                                   name="yt")
                    nc.vector.tensor_add(yt, py, b2t)
                    nc.sync.dma_start(
                        out=y[it * 128 : (it + 1) * 128, :], in_=yt
                    )

    nc.finalize()
    return nc


def _get_nc():
    if "nc" not in _CACHED:
        _CACHED["nc"] = _build()
    return _CACHED["nc"]


def _prep_in_maps(x, W1, b1, Wq, bq, Wk, bk, Wv, bv, a, W2, b2):
    f32 = np.float32
    x = np.asarray(x, f32)

    def wmat(W, dtype, ktiles, scale=1.0):
        W = np.asarray(W, f32) * f32(scale)
        return np.ascontiguousarray(
            W.reshape(ktiles, 128, W.shape[1]).transpose(1, 0, 2)
        ).astype(dtype)

    def bcol(b):
        return np.ascontiguousarray(np.asarray(b, f32).reshape(2, 128).T)

    av = f32(np.asarray(a, f32).reshape(-1)[0])
    shared = {
        "w1": wmat(W1, f32, 4),
        "w1b": wmat(W1, BF16, 4),
        "wq": wmat(Wq, BF16, 2),
        "wk": wmat(Wk, BF16, 2),
        "wv": wmat(Wv, BF16, 2),
        # 1/VSCALE un-does the fp8-range shim applied to v' (see module doc)
        "w2": wmat(W2, f32, 2, scale=1.0 / VSCALE),
        "b1c": bcol(b1),
        "bqc": bcol(bq),
        "bkc": bcol(bk),
        "bvb": np.ascontiguousarray(
            np.broadcast_to(np.asarray(bv, f32), (128, C))
        ),
        "b2b": np.ascontiguousarray(
            np.broadcast_to(np.asarray(b2, f32), (128, C))
        ),
        "acol": np.full((128, 1), av * f32(VSCALE), f32),
        "omac": np.full((128, 1), (f32(1.0) - av) * f32(VSCALE), f32),
    }
    maps = []
    for r in range(NCORES):
        xr = x[r * NL : (r + 1) * NL]
        xT3 = np.ascontiguousarray(xr.T.reshape(4, 128, NL).transpose(1, 0, 2))
        maps.append({**shared, "xT": xT3, "xTb": xT3.astype(BF16)})
    return maps


def kernel(**inputs) -> np.ndarray:
    from concourse.bass_utils import run_bass_kernel_spmd

    nc = _get_nc()
    in_maps = _prep_in_maps(**inputs)
    res = run_bass_kernel_spmd(nc, in_maps, list(range(NCORES)))
    return np.ascontiguousarray(
        np.concatenate(
            [res.results[r]["y"] for r in range(NCORES)], axis=0
        ).astype(np.float32)
    )



# revision 13
# speedup vs baseline: 1.9126x; 1.9126x over previous
"""Trainium2 Bass kernel for gated dense attention with dim=0 softmax.

Computation (reference):
    h = x @ W1 + b1
    q,k,v = h @ W{q,k,v} + b{q,k,v}
    w = (q @ k.T) / sqrt(256)
    attn = softmax(w, axis=0)          # normalizes over ROWS per column
    h2 = a*h + (1-a)*(attn @ v)
    out = h2 @ W2 + b2

Distribution strategy (chosen for the cost model's collective pricing:
every collective costs a flat ~15us + bytes/40GBps, AllReduce x1.875):

  Replicate x to all 8 cores as fp8 (4MB HBM->SBUF stream, ~12us of DMA
  at 360GB/s, far cheaper than a 2MB AllGather at collective rates).
  Core r owns COLUMN block J_r = [r*1024, (r+1)*1024) of the attention
  matrix: it computes q for ALL rows locally (x replicated), k,v only
  for its local rows, then S^r = k_r @ q_all^T -> [1024 j, 8192 i].
  The dim=0 softmax denominator sums over ALL i -- fully local in this
  layout (no stats AllReduce at all).  Each core then forms the partial
  output u^r[i,:] = sum_{j in J_r} P[i,j] v'[j,:] for ALL i, and a
  single bf16 ReduceScatter (512KB out, ~28us) both sums the partials
  over cores and hands each core exactly its own 1024 rows.

  Collectives: ONE ReduceScatter.  (Baseline: AG-k + AG-v + 4 stats
  AllReduces ~ 249us of serialized collective time.)

Weight folding (host, O(weights) only): q = x @ (W1@Wq) + (b1@Wq + bq),
same for k,v; the residual+projection y = a*h@W2 + (1-a)*u@W2 + b2 is
computed as x @ (a*W1@W2) + u'' @ (W2/VS) + (a*b1@W2 + b2) where
u'' = VS*(1-a)*u absorbs the softmax denominator via the per-j scale
g_j = (1-a)*VS / colsum_j folded into v.

Precision plan: attention path in fp8 (x, folded Wq/Wk scaled by 16 for
fp8 range; exp scale 1/(16*16*16) restores the 1/sqrt(H) temperature),
DoubleRow matmuls at 0.5 cyc/row.  exp on ScalarE with accum_out
producing column sums (the pacing engine: ~66us for 8.4M exps/core).
Partials and final projection in bf16; ReduceScatter accumulates fp32.
"""

import numpy as np
import ml_dtypes

BF16 = ml_dtypes.bfloat16
FP8 = ml_dtypes.float8_e4m3fn

N, D, H, C = 8192, 512, 256, 256
NCORES = 8
NL = N // NCORES          # 1024 rows/columns per core
JT = NL // 128            # 8 local j-tiles
NPAIR = JT // 2           # 4 DoubleRow j-pairs
NIC = N // 2048           # 4 i-chunks at the 2048-wide exp grain
WS = 16.0                 # fp8 range scale on folded Wq/Wk
VS = 8192.0               # fp8/bf16 range shim on v' and u''
EXPSCALE = 1.0 / (WS * WS * 16.0)   # restores exp(q.k/sqrt(256))
NBP = 5 + 2 * C           # packed bias columns

_CACHED = {}


def _build():
    import concourse.mybir as mybir
    from concourse import bacc
    from concourse.tile import TileContext

    dt = mybir.dt
    AF = mybir.ActivationFunctionType
    DR = mybir.MatmulPerfMode.DoubleRow
    f32, bf, f8 = dt.float32, dt.bfloat16, dt.float8e4
    RG = [list(range(NCORES))]

    nc = bacc.Bacc(None, target_bir_lowering=False, num_devices=NCORES)

    # ---------------- I/O (per core) ----------------
    xT8 = nc.declare_dram_parameter("xT8", [128, 4, N], f8, isOutput=False)
    xTl8 = nc.declare_dram_parameter("xTl8", [128, 4, NL], f8, isOutput=False)
    xTlb = nc.declare_dram_parameter("xTlb", [128, 4, NL], bf, isOutput=False)
    wq8 = nc.declare_dram_parameter("wq8", [128, 4, H], f8, isOutput=False)
    wk8 = nc.declare_dram_parameter("wk8", [128, 4, H], f8, isOutput=False)
    wv8 = nc.declare_dram_parameter("wv8", [128, 4, H], f8, isOutput=False)
    w12b = nc.declare_dram_parameter("w12b", [128, 4, C], bf, isOutput=False)
    w2b = nc.declare_dram_parameter("w2b", [128, 2, C], bf, isOutput=False)
    bpk = nc.declare_dram_parameter("bpk", [128, NBP], f32, isOutput=False)
    y = nc.declare_dram_parameter("y", [NL, C], f32, isOutput=True)

    # partial outputs, laid out [rank, ct, cp, i'] so ReduceScatter hands
    # rank r exactly u''^T for its own rows (lhsT-ready for the final mm)
    pbuf = nc.dram_tensor("pbuf", [NCORES * 2 * 128 * NL], bf)
    rsout = nc.dram_tensor("rsout", [2 * 128 * NL], bf)

    with TileContext(nc) as tc:
        with (
            tc.tile_pool(name="cst", bufs=1) as cst,
            tc.tile_pool(name="big", bufs=1) as big,
            tc.tile_pool(name="strm", bufs=1) as strm,
            tc.tile_pool(name="psum", bufs=1, space="PSUM") as psum,
        ):
            # ---- SBUF residents ----
            bpt = cst.tile([128, NBP], f32, tag="bpt", name="bpt")
            wqt = cst.tile([128, 4, H], f8, tag="wqt", name="wqt")
            wkt = cst.tile([128, 4, H], f8, tag="wkt", name="wkt")
            wvt = cst.tile([128, 4, H], f8, tag="wvt", name="wvt")
            w12t = cst.tile([128, 4, C], bf, tag="w12t", name="w12t")
            w2t = cst.tile([128, 2, C], bf, tag="w2t", name="w2t")
            scr = cst.tile([128, 1], f32, tag="scr", name="scr")

            xt8 = big.tile([128, 4, N], f8, tag="xt8", name="xt8")
            xtl8 = big.tile([128, 4, NL], f8, tag="xtl8", name="xtl8")
            xtlb = big.tile([128, 4, NL], bf, tag="xtlb", name="xtlb")
            qts = big.tile([128, 2, N], f8, tag="qts", name="qts")
            kts = big.tile([128, 2, NL], f8, tag="kts", name="kts")
            vts = [big.tile([128, 2, C], f8, tag=f"vts{t}", name=f"vts{t}")
                   for t in range(NPAIR)]
            vss = [big.tile([128, 2, C], f8, tag=f"vss{t}", name=f"vss{t}")
                   for t in range(NPAIR)]
            pts = [big.tile([128, 2, N], f8, tag=f"pts{t}", name=f"pts{t}")
                   for t in range(NPAIR)]
            colsq = big.tile([128, 4 * JT], f32, tag="colsq", name="colsq")
            csum = big.tile([128, JT], f32, tag="csum", name="csum")
            ginv = big.tile([128, JT], f32, tag="ginv", name="ginv")
            uts = big.tile([128, 2, NL], bf, tag="uts", name="uts")

            bqc = lambda hm: bpt[:, hm:hm + 1]
            bkc = lambda hm: bpt[:, 2 + hm:3 + hm]
            gsc = bpt[:, 4:5]
            bvt = bpt[:, 5:5 + C]
            b2t = bpt[:, 5 + C:5 + 2 * C]

            # ---- ACT table preload: get the Exp table load off the
            # critical path before the first real exp
            nc.vector.memset(scr, 0.0)
            nc.scalar.activation(scr, scr, AF.Exp, scale=0.0)

            # ---- input DMAs ----
            # ACT queue: small weights needed first (biases, Wq, Wk)
            nc.scalar.dma_start(out=bpt, in_=bpk[:])
            nc.scalar.dma_start(out=wqt, in_=wq8[:])
            nc.scalar.dma_start(out=wkt, in_=wk8[:])
            # gpsimd queue: local x slice (k,v inputs) + Wv
            nc.gpsimd.dma_start(out=xtl8, in_=xTl8[:])
            nc.gpsimd.dma_start(out=wvt, in_=wv8[:])
            # SP queue: the big replicated-x stream, one chunk per i-2048
            for c in range(NIC):
                sl = slice(c * 2048, (c + 1) * 2048)
                nc.sync.dma_start(out=xt8[:, :, sl], in_=xT8[:, :, sl])
            # final-phase constants (needed ~60us in; issued here on SP,
            # transfers slot in behind the x stream)
            nc.sync.dma_start(out=xtlb, in_=xTlb[:])
            nc.sync.dma_start(out=w12t, in_=w12b[:])
            nc.sync.dma_start(out=w2t, in_=w2b[:])

            # engine round-robin for psum->sbuf conversions
            conv_engs = [nc.gpsimd, nc.vector, nc.scalar]
            conv_i = [0]

            def conv_add(out, in_, scalar):
                e = conv_engs[conv_i[0] % 3]
                conv_i[0] += 1
                if e is nc.scalar:
                    nc.scalar.activation(out, in_, AF.Identity, bias=scalar)
                else:
                    e.tensor_scalar_add(out, in_, scalar)

            # ---- head: k, q (all rows), v ----
            # k^T[hq, j_local]: 4 [128,512] slices (hm, jc) of one psum tile
            kp = psum.tile([128, 2048], f32, tag="S", bufs=2, name="kp")
            for hm in range(2):
                for jc in range(2):
                    for u in range(2):
                        nc.tensor.matmul(
                            kp[:, (hm * 2 + jc) * 512:(hm * 2 + jc + 1) * 512],
                            lhsT=wkt[:, 2 * u:2 * u + 2, hm * 128:(hm + 1) * 128],
                            rhs=xtl8[:, 2 * u:2 * u + 2, jc * 512:(jc + 1) * 512],
                            start=(u == 0), stop=(u == 1), perf_mode=DR,
                        )
            for hm in range(2):
                conv_add(kts[:, hm, :], kp[:, hm * 1024:(hm + 1) * 1024], bkc(hm))

            # q^T[hq, i] for ALL i, chunked to follow the x stream
            for c in range(NIC):
                for hm in range(2):
                    qp = psum.tile([128, 2048], f32, tag="S", bufs=2, name="qp")
                    for u in range(2):
                        for s in range(4):
                            nc.tensor.matmul(
                                qp[:, s * 512:(s + 1) * 512],
                                lhsT=wqt[:, 2 * u:2 * u + 2, hm * 128:(hm + 1) * 128],
                                rhs=xt8[:, 2 * u:2 * u + 2,
                                        c * 2048 + s * 512:c * 2048 + (s + 1) * 512],
                                start=(u == 0), stop=(u == 1), perf_mode=DR,
                            )
                    conv_add(qts[:, hm, c * 2048:(c + 1) * 2048], qp, bqc(hm))

            # v[j_local, c]: 8 j-tiles into pair tiles
            vp = psum.tile([128, 2048], f32, tag="S", bufs=2, name="vp")
            for jt in range(JT):
                for u in range(2):
                    nc.tensor.matmul(
                        vp[:, jt * 256:(jt + 1) * 256],
                        lhsT=xtl8[:, 2 * u:2 * u + 2, jt * 128:(jt + 1) * 128],
                        rhs=wvt[:, 2 * u:2 * u + 2, :],
                        start=(u == 0), stop=(u == 1), perf_mode=DR,
                    )
            for t in range(NPAIR):
                for jm in range(2):
                    e = [nc.gpsimd, nc.vector][(2 * t + jm) % 2]
                    e.tensor_add(vts[t][:, jm, :],
                                 vp[:, (2 * t + jm) * 256:(2 * t + jm + 1) * 256],
                                 bvt)

            # ---- S + exp phase: i-chunk outer so it starts on x chunk 0 ----
            for c in range(NIC):
                for jt in range(JT):
                    sp = psum.tile([128, 2048], f32, tag="S", bufs=2, name="sp")
                    for s in range(4):
                        nc.tensor.matmul(
                            sp[:, s * 512:(s + 1) * 512],
                            lhsT=kts[:, :, jt * 128:(jt + 1) * 128],
                            rhs=qts[:, :, c * 2048 + s * 512:c * 2048 + (s + 1) * 512],
                            start=True, stop=True, perf_mode=DR,
                        )
                    nc.scalar.activation(
                        pts[jt // 2][:, jt % 2, c * 2048:(c + 1) * 2048],
                        sp, AF.Exp, scale=EXPSCALE,
                        accum_out=colsq[:, jt * 4 + c:jt * 4 + c + 1],
                    )
                    if c == NIC - 1:
                        # last i-chunk for this j-tile: finish the column sum,
                        # fold (1-a)*VS/colsum into a per-j scale
                        nc.vector.tensor_reduce(
                            csum[:, jt:jt + 1],
                            colsq[:, jt * 4:(jt + 1) * 4],
                            mybir.AxisListType.X,
                            mybir.AluOpType.add,
                        )
                        nc.vector.reciprocal(ginv[:, jt:jt + 1], csum[:, jt:jt + 1])
                        nc.vector.tensor_scalar_mul(
                            ginv[:, jt:jt + 1], ginv[:, jt:jt + 1], gsc)
                        if jt % 2 == 1:
                            t = jt // 2
                            for jm in range(2):
                                nc.gpsimd.tensor_scalar_mul(
                                    vss[t][:, jm, :], vts[t][:, jm, :],
                                    ginv[:, 2 * t + jm:2 * t + jm + 1])

            # ---- PV phase: u''^T[c, i] partials, drained bf16 to pbuf ----
            # pbuf flat layout [r, cp, ct, i']: the RS shard for rank r is
            # [cp, ct, i'] = u''^T for its own 1024 rows, lhsT-ready.
            drain_engs = [nc.gpsimd, nc.scalar, nc.vector]
            for g in range(4):
                stg = strm.tile([128, 2, 2, NL], bf, tag="stg", bufs=2,
                                name=f"stg{g}")  # [cp, r', ct, i']
                for ct in range(2):
                    pv = psum.tile([128, 2, NL], f32, tag="S", bufs=2,
                                   name="pv")    # [cp, r', i']
                    for sub in range(4):
                        ic = g * 4 + sub
                        for t in range(NPAIR - 1, -1, -1):
                            nc.tensor.matmul(
                                pv[:, sub // 2, (sub % 2) * 512:(sub % 2 + 1) * 512],
                                lhsT=vss[t][:, :, ct * 128:(ct + 1) * 128],
                                rhs=pts[t][:, :, ic * 512:(ic + 1) * 512],
                                start=(t == NPAIR - 1), stop=(t == 0),
                                perf_mode=DR,
                            )
                    e = drain_engs[(g * 2 + ct) % 3]
                    if e is nc.scalar:
                        nc.scalar.copy(stg[:, :, ct, :], pv)
                    else:
                        e.tensor_scalar_add(stg[:, :, ct, :], pv, 0.0)
                off = g * (2 * 2 * 128 * NL)
                nc.sync.dma_start(
                    out=pbuf[off:off + 2 * 2 * 128 * NL].rearrange(
                        "(r p c i) -> p r c i", r=2, p=128, c=2),
                    in_=stg,
                )

            # ---- single collective: sum partials, scatter rows to owners ----
            nc.gpsimd.collective_compute(
                "ReduceScatter",
                mybir.AluOpType.add,
                replica_groups=RG,
                ins=[pbuf[:]],
                outs=[rsout[:]],
            )

            # ---- final: y = x@(a W1W2) + u''@(W2/VS) + b'' ----
            # x-term runs DURING the ReduceScatter (PE otherwise idle);
            # converted to SBUF with b'' folded in, then added to the u-term
            xyt = big.tile([128, 8, C], f32, tag="xyt", name="xyt")
            fx = psum.tile([128, 2048], f32, tag="S", bufs=2, name="fx")
            for it in range(8):
                for u in range(4):
                    nc.tensor.matmul(
                        fx[:, it * 256:(it + 1) * 256],
                        lhsT=xtlb[:, u, it * 128:(it + 1) * 128],
                        rhs=w12t[:, u, :],
                        start=(u == 0), stop=(u == 3),
                    )
            for it in range(8):
                e = [nc.vector, nc.gpsimd][it % 2]
                e.tensor_add(xyt[:, it, :],
                             fx[:, it * 256:(it + 1) * 256], b2t)
            nc.sync.dma_start(
                out=uts,
                in_=rsout[:].rearrange("(p c i) -> p c i", p=128, c=2),
            )
            fy = psum.tile([128, 2048], f32, tag="S", bufs=2, name="fy")
            for b in range(2):
                yst = strm.tile([128, 4, C], f32, tag="yst", bufs=2,
                                name=f"yst{b}")
                for q in range(4):
                    it = b * 4 + q
                    for ct in range(2):
                        nc.tensor.matmul(
                            fy[:, it * 256:(it + 1) * 256],
                            lhsT=uts[:, ct, it * 128:(it + 1) * 128],
                            rhs=w2t[:, ct, :],
                            start=(ct == 0), stop=(ct == 1),
                        )
                    e = [nc.vector, nc.gpsimd][it % 2]
                    e.tensor_add(yst[:, q, :],
                                 fy[:, it * 256:(it + 1) * 256],
                                 xyt[:, it, :])
                nc.sync.dma_start(
                    out=y[b * 512:(b + 1) * 512, :].rearrange(
                        "(a p) c -> p a c", a=4, p=128),
                    in_=yst,
                )

    nc.finalize()
    return nc


def _get_nc():
    if "nc" not in _CACHED:
        _CACHED["nc"] = _build()
    return _CACHED["nc"]


def _prep_in_maps(x, W1, b1, Wq, bq, Wk, bk, Wv, bv, a, W2, b2):
    f32 = np.float32
    x = np.asarray(x, f32)
    W1 = np.asarray(W1, f32)
    b1 = np.asarray(b1, f32)
    av = f32(np.asarray(a, f32).reshape(-1)[0])

    def fold(Wx, bx):
        Wx, bx = np.asarray(Wx, f32), np.asarray(bx, f32)
        return W1 @ Wx, b1 @ Wx + bx

    Wqf, bqf = fold(Wq, bq)
    Wkf, bkf = fold(Wk, bk)
    Wvf, bvf = fold(Wv, bv)
    W2_ = np.asarray(W2, f32)
    W12 = av * (W1 @ W2_)
    b12 = av * (b1 @ W2_) + np.asarray(b2, f32)

    def pack(W, dtype, kt):
        return np.ascontiguousarray(
            W.reshape(kt, 128, W.shape[1]).transpose(1, 0, 2)).astype(dtype)

    def bcol(v2):
        return np.ascontiguousarray(v2.reshape(2, 128).T)

    bp = np.zeros((128, NBP), f32)
    bp[:, 0:2] = bcol(bqf * f32(WS))
    bp[:, 2:4] = bcol(bkf * f32(WS))
    bp[:, 4] = (f32(1.0) - av) * f32(VS)
    bp[:, 5:5 + C] = np.broadcast_to(bvf, (128, C))
    bp[:, 5 + C:5 + 2 * C] = np.broadcast_to(b12, (128, C))

    xT = np.ascontiguousarray(x.T.reshape(4, 128, N).transpose(1, 0, 2))
    xT8 = xT.astype(FP8)

    shared = {
        "xT8": xT8,
        "wq8": pack(Wqf * f32(WS), FP8, 4),
        "wk8": pack(Wkf * f32(WS), FP8, 4),
        "wv8": pack(Wvf, FP8, 4),
        "w12b": pack(W12, BF16, 4),
        "w2b": pack(W2_ * f32(1.0 / VS), BF16, 2),
        "bpk": bp,
    }
    maps = []
    for r in range(NCORES):
        sl = slice(r * NL, (r + 1) * NL)
        maps.append({
            **shared,
            "xTl8": np.ascontiguousarray(xT8[:, :, sl]),
            "xTlb": np.ascontiguousarray(xT[:, :, sl]).astype(BF16),
        })
    return maps


def kernel(**inputs) -> np.ndarray:
    from concourse.bass_utils import run_bass_kernel_spmd

    nc = _get_nc()
    in_maps = _prep_in_maps(**inputs)
    res = run_bass_kernel_spmd(nc, in_maps, list(range(NCORES)))
    return np.ascontiguousarray(
        np.concatenate(
            [res.results[r]["y"] for r in range(NCORES)], axis=0
        ).astype(np.float32)
    )


# revision 16
# speedup vs baseline: 2.0965x; 1.0961x over previous
"""Trainium2 Bass kernel for gated dense attention with dim=0 softmax.

Computation (reference):
    h = x @ W1 + b1
    q,k,v = h @ W{q,k,v} + b{q,k,v}
    w = (q @ k.T) / sqrt(256)
    attn = softmax(w, axis=0)          # normalizes over ROWS per column
    h2 = a*h + (1-a)*(attn @ v)
    out = h2 @ W2 + b2

Distribution strategy (chosen for the cost model's collective pricing:
every collective costs a flat ~15us + bytes/40GBps, AllReduce x1.875):

  Replicate x to all 8 cores as fp8 (4MB HBM->SBUF stream at 360GB/s is
  far cheaper than any AllGather at collective rates).  Core r owns
  COLUMN block J_r = [r*1024, (r+1)*1024) of the attention matrix: it
  computes q for ALL rows locally (x replicated), k,v only for its own
  rows, then S^r = k_r @ q_all^T -> [1024 j, 8192 i].  The dim=0
  softmax denominator sums over ALL i -- fully local in this layout (no
  stats AllReduce).  Each core forms the partial output
  u^r[i,:] = sum_{j in J_r} P[i,j] v'[j,:] for ALL i, and a single fp8
  ReduceScatter (256KB out, ~21.5us; reduction itself runs fp32) both
  sums the partials over cores and hands each core its own 1024 rows.

  Collectives: ONE ReduceScatter.  (Baseline: AG-k + AG-v + 4 stats
  AllReduces ~ 249us of serialized collective time.)

Schedule: the pacing engine is ScalarE doing the 8.4M exps/core.  exp
runs at [128,2048] grain from a 2-deep psum ping-pong, back-to-back at
1892ns; S matmuls (fp8 DoubleRow, ~1.8us/chunk) hide underneath, and
q-chunk production is interleaved INTO the S stream so the first exp
fires ~11us in.  Column sums run on the otherwise-idle Pool engine
(tensor_reduce over the just-written fp8 P tiles) except the last
i-chunk, which uses the exp's accum_out so the per-j scale is ready
the moment its final exp retires.  During the ReduceScatter the PE
computes the residual x@(a*W1W2) term and then chews dummy f32 matmuls
to stay at full clock for the post-collective projection.

Weight folding (host, O(weights) only): q = x @ (W1@Wq) + (b1@Wq + bq),
same for k,v; y = x @ (a*W1@W2) + u'' @ (8*W2) / (8*VS) + (a*b1@W2+b2)
with u'' = VS*(1-a)*u carried through the fp8 ReduceScatter.
"""

import numpy as np
import ml_dtypes

BF16 = ml_dtypes.bfloat16
FP8 = ml_dtypes.float8_e4m3fn

N, D, H, C = 8192, 512, 256, 256
NCORES = 8
NL = N // NCORES          # 1024 rows/columns per core
JT = NL // 128            # 8 local j-tiles
NPAIR = JT // 2           # 4 DoubleRow j-pairs
NIC = N // 2048           # 4 i-chunks at the 2048-wide exp grain
WS = 16.0                 # fp8 range scale on folded Wq/Wk
VS = 4096.0               # range shim on v' and u'' (u'' stays in fp8 range)
W2S = 8.0                 # fp8 range scale on W2
EXPSCALE = 1.0 / (WS * WS * 16.0)   # restores exp(q.k/sqrt(256))
NBP = 5 + 2 * C           # packed bias columns
NDUMMY = 34               # f32 warm-up matmuls spanning the ReduceScatter

_CACHED = {}


def _build():
    import concourse.mybir as mybir
    from concourse import bacc
    from concourse.tile import TileContext

    dt = mybir.dt
    AF = mybir.ActivationFunctionType
    DR = mybir.MatmulPerfMode.DoubleRow
    f32, bf, f8 = dt.float32, dt.bfloat16, dt.float8e4
    RG = [list(range(NCORES))]

    nc = bacc.Bacc(None, target_bir_lowering=False, num_devices=NCORES)

    # ---------------- I/O (per core) ----------------
    xT8 = nc.declare_dram_parameter("xT8", [128, 4, N], f8, isOutput=False)
    xTl8 = nc.declare_dram_parameter("xTl8", [128, 4, NL], f8, isOutput=False)
    xTlb = nc.declare_dram_parameter("xTlb", [128, 4, NL], bf, isOutput=False)
    wq8 = nc.declare_dram_parameter("wq8", [128, 4, H], f8, isOutput=False)
    wk8 = nc.declare_dram_parameter("wk8", [128, 4, H], f8, isOutput=False)
    wv8 = nc.declare_dram_parameter("wv8", [128, 4, H], f8, isOutput=False)
    w12b = nc.declare_dram_parameter("w12b", [128, 4, C], bf, isOutput=False)
    w2f = nc.declare_dram_parameter("w2f", [128, 2, C], f8, isOutput=False)
    bpk = nc.declare_dram_parameter("bpk", [128, NBP], f32, isOutput=False)
    y = nc.declare_dram_parameter("y", [NL, C], f32, isOutput=True)

    # partial outputs, laid out [rank][cp, ct, i'] so the ReduceScatter
    # shard for rank r is u''^T for its own rows, lhsT-ready
    pbuf = nc.dram_tensor("pbuf", [NCORES * 2 * 128 * NL], f8)
    rsout = nc.dram_tensor("rsout", [2 * 128 * NL], f8)

    with TileContext(nc) as tc:
        with (
            tc.tile_pool(name="cst", bufs=1) as cst,
            tc.tile_pool(name="big", bufs=1) as big,
            tc.tile_pool(name="strm", bufs=1) as strm,
            tc.tile_pool(name="psum", bufs=1, space="PSUM") as psum,
        ):
            # ---- SBUF residents ----
            bpt = cst.tile([128, NBP], f32, tag="bpt", name="bpt")
            wqt = cst.tile([128, 4, H], f8, tag="wqt", name="wqt")
            wkt = cst.tile([128, 4, H], f8, tag="wkt", name="wkt")
            wvt = cst.tile([128, 4, H], f8, tag="wvt", name="wvt")
            w12t = cst.tile([128, 4, C], bf, tag="w12t", name="w12t")
            w2t = cst.tile([128, 2, C], f8, tag="w2t", name="w2t")
            scr = cst.tile([128, 1], f32, tag="scr", name="scr")

            xt8 = big.tile([128, 4, N], f8, tag="xt8", name="xt8")
            xtl8 = big.tile([128, 4, NL], f8, tag="xtl8", name="xtl8")
            xtlb = big.tile([128, 4, NL], bf, tag="xtlb", name="xtlb")
            qts = big.tile([128, 2, N], f8, tag="qts", name="qts")
            kts = big.tile([128, 2, NL], f8, tag="kts", name="kts")
            vts = [big.tile([128, 2, C], f8, tag=f"vts{t}", name=f"vts{t}")
                   for t in range(NPAIR)]
            vss = [big.tile([128, 2, C], f8, tag=f"vss{t}", name=f"vss{t}")
                   for t in range(NPAIR)]
            pts = [big.tile([128, 2, N], f8, tag=f"pts{t}", name=f"pts{t}")
                   for t in range(NPAIR)]
            colsq = big.tile([128, 4 * JT], f32, tag="colsq", name="colsq")
            csum = big.tile([128, JT], f32, tag="csum", name="csum")
            ginv = big.tile([128, JT], f32, tag="ginv", name="ginv")
            uts = big.tile([128, 2, NL], f8, tag="uts", name="uts")
            xyt = big.tile([128, 8, C], f32, tag="xyt", name="xyt")

            bqc = lambda hm: bpt[:, hm:hm + 1]
            bkc = lambda hm: bpt[:, 2 + hm:3 + hm]
            gsc = bpt[:, 4:5]
            bvt = bpt[:, 5:5 + C]
            b2t = bpt[:, 5 + C:5 + 2 * C]

            # ---- ACT table preload: Exp table load off the critical path
            nc.vector.memset(scr, 0.0)
            nc.scalar.activation(scr, scr, AF.Exp, scale=0.0)

            # ---- input DMAs ----
            nc.scalar.dma_start(out=bpt, in_=bpk[:])
            nc.scalar.dma_start(out=wqt, in_=wq8[:])
            nc.scalar.dma_start(out=wkt, in_=wk8[:])
            nc.gpsimd.dma_start(out=xtl8, in_=xTl8[:])
            nc.gpsimd.dma_start(out=wvt, in_=wv8[:])
            for c in range(NIC):
                sl = slice(c * 2048, (c + 1) * 2048)
                nc.sync.dma_start(out=xt8[:, :, sl], in_=xT8[:, :, sl])
            nc.sync.dma_start(out=xtlb, in_=xTlb[:])
            nc.sync.dma_start(out=w12t, in_=w12b[:])
            nc.sync.dma_start(out=w2t, in_=w2f[:])

            # ---- PE helpers ----
            def q_mms(c):
                """q^T[hq, i-chunk c] -> qts (convert on Pool for c0/hm0
                while Pool is still idle, DVE otherwise)."""
                for hm in range(2):
                    qp = psum.tile([128, 2048], f32, tag="S", bufs=2,
                                   name="qp")
                    for u in range(2):
                        for s in range(4):
                            nc.tensor.matmul(
                                qp[:, s * 512:(s + 1) * 512],
                                lhsT=wqt[:, 2 * u:2 * u + 2,
                                         hm * 128:(hm + 1) * 128],
                                rhs=xt8[:, 2 * u:2 * u + 2,
                                        c * 2048 + s * 512:c * 2048 + (s + 1) * 512],
                                start=(u == 0), stop=(u == 1), perf_mode=DR,
                            )
                    e = nc.gpsimd if (c == 0 and hm == 0) else nc.vector
                    e.tensor_scalar_add(
                        qts[:, hm, c * 2048:(c + 1) * 2048], qp, bqc(hm))

            def v_mms():
                vp = psum.tile([128, 2048], f32, tag="S", bufs=2, name="vp")
                for jt in range(JT):
                    for u in range(2):
                        nc.tensor.matmul(
                            vp[:, jt * 256:(jt + 1) * 256],
                            lhsT=xtl8[:, 2 * u:2 * u + 2,
                                      jt * 128:(jt + 1) * 128],
                            rhs=wvt[:, 2 * u:2 * u + 2, :],
                            start=(u == 0), stop=(u == 1), perf_mode=DR,
                        )
                for t in range(NPAIR):
                    for jm in range(2):
                        nc.vector.tensor_add(
                            vts[t][:, jm, :],
                            vp[:, (2 * t + jm) * 256:(2 * t + jm + 1) * 256],
                            bvt)

            # ---- head: k then q chunk 0 ----
            kp = psum.tile([128, 2048], f32, tag="S", bufs=2, name="kp")
            for hm in range(2):
                for jc in range(2):
                    for u in range(2):
                        nc.tensor.matmul(
                            kp[:, (hm * 2 + jc) * 512:(hm * 2 + jc + 1) * 512],
                            lhsT=wkt[:, 2 * u:2 * u + 2, hm * 128:(hm + 1) * 128],
                            rhs=xtl8[:, 2 * u:2 * u + 2, jc * 512:(jc + 1) * 512],
                            start=(u == 0), stop=(u == 1), perf_mode=DR,
                        )
            nc.gpsimd.tensor_scalar_add(kts[:, 0, :], kp[:, 0:1024], bkc(0))
            nc.vector.tensor_scalar_add(kts[:, 1, :], kp[:, 1024:2048], bkc(1))
            q_mms(0)

            # ---- S + exp phase: i-chunk outer; q for chunk c+1 and the v
            # production are interleaved into the S stream so psum buffer
            # rotation never stalls the exp pipeline
            for c in range(NIC):
                for jt in range(JT):
                    sp = psum.tile([128, 2048], f32, tag="S", bufs=2,
                                   name="sp")
                    for s in range(4):
                        nc.tensor.matmul(
                            sp[:, s * 512:(s + 1) * 512],
                            lhsT=kts[:, :, jt * 128:(jt + 1) * 128],
                            rhs=qts[:, :, c * 2048 + s * 512:c * 2048 + (s + 1) * 512],
                            start=True, stop=True, perf_mode=DR,
                        )
                    pslice = pts[jt // 2][:, jt % 2, c * 2048:(c + 1) * 2048]
                    col = colsq[:, jt * 4 + c:jt * 4 + c + 1]
                    nc.scalar.activation(pslice, sp, AF.Exp,
                                         scale=EXPSCALE, accum_out=col)
                    if c == NIC - 1:
                        nc.vector.tensor_reduce(
                            csum[:, jt:jt + 1], colsq[:, jt * 4:(jt + 1) * 4],
                            mybir.AxisListType.X, mybir.AluOpType.add)
                        nc.vector.reciprocal(ginv[:, jt:jt + 1],
                                             csum[:, jt:jt + 1])
                        nc.vector.tensor_scalar_mul(
                            ginv[:, jt:jt + 1], ginv[:, jt:jt + 1], gsc)
                        if jt % 2 == 1:
                            t = jt // 2
                            for jm in range(2):
                                nc.vector.tensor_scalar_mul(
                                    vss[t][:, jm, :], vts[t][:, jm, :],
                                    ginv[:, 2 * t + jm:2 * t + jm + 1])
                    if c == 0 and jt == 1 and NIC > 1:
                        q_mms(1)
                    if c == 0 and jt == 3:
                        v_mms()
                    if 0 < c < NIC - 1 and jt == 1:
                        q_mms(c + 1)

            # ---- PV phase: u''^T[c, i] partials, drained fp8 to pbuf ----
            drain_engs = [nc.gpsimd, nc.scalar, nc.vector]
            for g in range(4):
                stg = strm.tile([128, 2, 2, NL], f8, tag="stg", bufs=2,
                                name=f"stg{g}")  # [cp, r', ct, i']
                for ct in range(2):
                    pv = psum.tile([128, 2, NL], f32, tag="S", bufs=2,
                                   name="pv")    # [cp, r', i']
                    for sub in range(4):
                        ic = g * 4 + sub
                        for t in range(NPAIR):
                            nc.tensor.matmul(
                                pv[:, sub // 2, (sub % 2) * 512:(sub % 2 + 1) * 512],
                                lhsT=vss[t][:, :, ct * 128:(ct + 1) * 128],
                                rhs=pts[t][:, :, ic * 512:(ic + 1) * 512],
                                start=(t == 0), stop=(t == NPAIR - 1),
                                perf_mode=DR,
                            )
                    e = drain_engs[(g * 2 + ct) % 3]
                    if e is nc.scalar:
                        nc.scalar.copy(stg[:, :, ct, :], pv)
                    else:
                        e.tensor_scalar_add(stg[:, :, ct, :], pv, 0.0)
                    gb = 2 * 2 * 128 * NL
                    nc.sync.dma_start(
                        out=pbuf[g * gb:(g + 1) * gb].rearrange(
                            "(r p c i) -> p r c i", r=2, p=128, c=2)[:, :, ct, :],
                        in_=stg[:, :, ct, :],
                    )

            # ---- single collective: sum partials, scatter rows to owners ----
            nc.gpsimd.collective_compute(
                "ReduceScatter",
                mybir.AluOpType.add,
                replica_groups=RG,
                ins=[pbuf[:]],
                outs=[rsout[:]],
            )

            # ---- final: y = x@(a W1W2) + u''@(8 W2)/(8 VS) + b'' ----
            # x-term + PE warm-up dummies run DURING the ReduceScatter
            fx = psum.tile([128, 2048], f32, tag="S", bufs=2, name="fx")
            for it in range(8):
                for u in range(4):
                    nc.tensor.matmul(
                        fx[:, it * 256:(it + 1) * 256],
                        lhsT=xtlb[:, u, it * 128:(it + 1) * 128],
                        rhs=w12t[:, u, :],
                        start=(u == 0), stop=(u == 3),
                    )
            for it in range(8):
                e = [nc.vector, nc.gpsimd][it % 2]
                e.tensor_add(xyt[:, it, :],
                             fx[:, it * 256:(it + 1) * 256], b2t)
            nc.sync.dma_start(
                out=uts,
                in_=rsout[:].rearrange("(p c i) -> p c i", p=128, c=2),
            )
            # keep the PE p-state at full clock across the collective (slow
            # f32 matmuls into a scratch psum tile, no data deps on the RS)
            fdum = psum.tile([128, 2048], f32, tag="S", bufs=2, name="fdum")
            for d in range(NDUMMY):
                nc.tensor.matmul(
                    fdum[:, 0:512],
                    lhsT=bpt[:, 0:128],
                    rhs=bpt[:, 0:512],
                    start=True, stop=True,
                )
            fy = psum.tile([128, 2048], f32, tag="S", bufs=2, name="fy")
            for b in range(2):
                yst = strm.tile([128, 4, C], f32, tag="yst", bufs=2,
                                name=f"yst{b}")
                for qq in range(4):
                    it = b * 4 + qq
                    nc.tensor.matmul(
                        fy[:, it * 256:(it + 1) * 256],
                        lhsT=uts[:, :, it * 128:(it + 1) * 128],
                        rhs=w2t[:, :, :],
                        start=True, stop=True, perf_mode=DR,
                    )
                    e = [nc.vector, nc.gpsimd][it % 2]
                    e.scalar_tensor_tensor(
                        yst[:, qq, :],
                        fy[:, it * 256:(it + 1) * 256],
                        1.0 / (W2S * VS),
                        xyt[:, it, :],
                        op0=mybir.AluOpType.mult,
                        op1=mybir.AluOpType.add,
                    )
                nc.sync.dma_start(
                    out=y[b * 512:(b + 1) * 512, :].rearrange(
                        "(a p) c -> p a c", a=4, p=128),
                    in_=yst,
                )

    nc.finalize()
    return nc


def _get_nc():
    if "nc" not in _CACHED:
        _CACHED["nc"] = _build()
    return _CACHED["nc"]


def _prep_in_maps(x, W1, b1, Wq, bq, Wk, bk, Wv, bv, a, W2, b2):
    f32 = np.float32
    x = np.asarray(x, f32)
    W1 = np.asarray(W1, f32)
    b1 = np.asarray(b1, f32)
    av = f32(np.asarray(a, f32).reshape(-1)[0])

    def fold(Wx, bx):
        Wx, bx = np.asarray(Wx, f32), np.asarray(bx, f32)
        return W1 @ Wx, b1 @ Wx + bx

    Wqf, bqf = fold(Wq, bq)
    Wkf, bkf = fold(Wk, bk)
    Wvf, bvf = fold(Wv, bv)
    W2_ = np.asarray(W2, f32)
    W12 = av * (W1 @ W2_)
    b12 = av * (b1 @ W2_) + np.asarray(b2, f32)

    def pack(W, dtype, kt):
        return np.ascontiguousarray(
            W.reshape(kt, 128, W.shape[1]).transpose(1, 0, 2)).astype(dtype)

    def bcol(v2):
        return np.ascontiguousarray(v2.reshape(2, 128).T)

    bp = np.zeros((128, NBP), f32)
    bp[:, 0:2] = bcol(bqf * f32(WS))
    bp[:, 2:4] = bcol(bkf * f32(WS))
    bp[:, 4] = (f32(1.0) - av) * f32(VS)
    bp[:, 5:5 + C] = np.broadcast_to(bvf, (128, C))
    bp[:, 5 + C:5 + 2 * C] = np.broadcast_to(b12, (128, C))

    xT = np.ascontiguousarray(x.T.reshape(4, 128, N).transpose(1, 0, 2))
    xT8 = xT.astype(FP8)

    shared = {
        "xT8": xT8,
        "wq8": pack(Wqf * f32(WS), FP8, 4),
        "wk8": pack(Wkf * f32(WS), FP8, 4),
        "wv8": pack(Wvf, FP8, 4),
        "w12b": pack(W12, BF16, 4),
        "w2f": pack(W2_ * f32(W2S), FP8, 2),
        "bpk": bp,
    }
    maps = []
    for r in range(NCORES):
        sl = slice(r * NL, (r + 1) * NL)
        maps.append({
            **shared,
            "xTl8": np.ascontiguousarray(xT8[:, :, sl]),
            "xTlb": np.ascontiguousarray(xT[:, :, sl]).astype(BF16),
        })
    return maps


def kernel(**inputs) -> np.ndarray:
    from concourse.bass_utils import run_bass_kernel_spmd

    nc = _get_nc()
    in_maps = _prep_in_maps(**inputs)
    res = run_bass_kernel_spmd(nc, in_maps, list(range(NCORES)))
    return np.ascontiguousarray(
        np.concatenate(
            [res.results[r]["y"] for r in range(NCORES)], axis=0
        ).astype(np.float32)
    )


# revision 23
# speedup vs baseline: 2.2053x; 1.0519x over previous
"""Trainium2 Bass kernel for gated dense attention with dim=0 softmax.

Computation (reference):
    h = x @ W1 + b1
    q,k,v = h @ W{q,k,v} + b{q,k,v}
    w = (q @ k.T) / sqrt(256)
    attn = softmax(w, axis=0)          # normalizes over ROWS per column
    h2 = a*h + (1-a)*(attn @ v)
    out = h2 @ W2 + b2

Distribution strategy (chosen for the cost model's collective pricing:
every collective costs a flat ~15us + bytes/40GBps, AllReduce x1.875):

  Replicate x to all 8 cores as fp8 (4MB HBM->SBUF stream at 360GB/s is
  far cheaper than any AllGather at collective rates).  Core r owns
  COLUMN block J_r = [r*1024, (r+1)*1024) of the attention matrix: it
  computes q for ALL rows locally (x replicated), k,v only for its own
  rows, then S^r = k_r @ q_all^T -> [1024 j, 8192 i].  The dim=0
  softmax denominator sums over ALL i -- fully local in this layout (no
  stats AllReduce).  Each core forms the partial output
  u^r[i,:] = sum_{j in J_r} P[i,j] v'[j,:] for ALL i, and a single fp8
  ReduceScatter (256KB out, ~21.5us; reduction itself runs fp32) both
  sums the partials over cores and hands each core its own 1024 rows.

  Collectives: ONE ReduceScatter.  (Baseline: AG-k + AG-v + 4 stats
  AllReduces ~ 249us of serialized collective time.)

Schedule: the pacing engine is ScalarE doing the 8.4M exps/core.  exp
runs at [128,2048] grain from a 2-deep psum ping-pong, back-to-back at
1892ns; S matmuls (fp8 DoubleRow, ~1.8us/chunk) hide underneath, and
q-chunk production is interleaved INTO the S stream so the first exp
fires ~11us in.  Column sums run on the otherwise-idle Pool engine
(tensor_reduce over the just-written fp8 P tiles) except the last
i-chunk, which uses the exp's accum_out so the per-j scale is ready
the moment its final exp retires.  During the ReduceScatter the PE
computes the residual x@(a*W1W2) term and then chews dummy f32 matmuls
to stay at full clock for the post-collective projection.

Weight folding (host, O(weights) only): q = x @ (W1@Wq) + (b1@Wq + bq),
same for k,v; y = x @ (a*W1@W2) + u'' @ (8*W2) / (8*VS) + (a*b1@W2+b2)
with u'' = VS*(1-a)*u carried through the fp8 ReduceScatter.
"""

import numpy as np
import ml_dtypes

BF16 = ml_dtypes.bfloat16
FP8 = ml_dtypes.float8_e4m3fn

N, D, H, C = 8192, 512, 256, 256
NCORES = 8
NL = N // NCORES          # 1024 rows/columns per core
JT = NL // 128            # 8 local j-tiles
NPAIR = JT // 2           # 4 DoubleRow j-pairs
NIC = N // 2048           # 4 i-chunks at the 2048-wide exp grain
WS = 16.0                 # fp8 range scale on folded Wq/Wk
VS = 4096.0               # range shim on v' and u'' (u'' stays in fp8 range)
W2S = 8.0                 # fp8 range scale on W2
EXPSCALE = 1.0 / (WS * WS * 16.0)   # restores exp(q.k/sqrt(256))
NBP = 5 + 2 * C           # packed bias columns
NDUMMY = 25               # f32 warm-up matmuls spanning the ReduceScatter

_CACHED = {}


def _build():
    import concourse.mybir as mybir
    from concourse import bacc
    from concourse.tile import TileContext

    dt = mybir.dt
    AF = mybir.ActivationFunctionType
    DR = mybir.MatmulPerfMode.DoubleRow
    f32, bf, f8 = dt.float32, dt.bfloat16, dt.float8e4
    RG = [list(range(NCORES))]

    nc = bacc.Bacc(None, target_bir_lowering=False, num_devices=NCORES)

    # ---------------- I/O (per core) ----------------
    xT8 = nc.declare_dram_parameter("xT8", [128, 4, N], f8, isOutput=False)
    xTl8 = nc.declare_dram_parameter("xTl8", [128, 4, NL], f8, isOutput=False)
    xTlb = nc.declare_dram_parameter("xTlb", [128, 4, NL], bf, isOutput=False)
    wq8 = nc.declare_dram_parameter("wq8", [128, 4, H], f8, isOutput=False)
    wk8 = nc.declare_dram_parameter("wk8", [128, 4, H], f8, isOutput=False)
    wv8 = nc.declare_dram_parameter("wv8", [128, 4, H], f8, isOutput=False)
    w12b = nc.declare_dram_parameter("w12b", [128, 4, C], bf, isOutput=False)
    w2f = nc.declare_dram_parameter("w2f", [128, 2, C], f8, isOutput=False)
    bpk = nc.declare_dram_parameter("bpk", [128, NBP], f32, isOutput=False)
    y = nc.declare_dram_parameter("y", [NL, C], f32, isOutput=True)

    # partial outputs, laid out [rank][cp, ct, i'] so the ReduceScatter
    # shard for rank r is u''^T for its own rows, lhsT-ready
    pbuf = nc.dram_tensor("pbuf", [NCORES * 2 * 128 * NL], f8)
    rsout = nc.dram_tensor("rsout", [2 * 128 * NL], f8)

    with TileContext(nc) as tc:
        with (
            tc.tile_pool(name="cst", bufs=1) as cst,
            tc.tile_pool(name="big", bufs=1) as big,
            tc.tile_pool(name="strm", bufs=1) as strm,
            tc.tile_pool(name="psum", bufs=1, space="PSUM") as psum,
        ):
            # ---- SBUF residents ----
            bpt = cst.tile([128, NBP], f32, tag="bpt", name="bpt")
            wqt = cst.tile([128, 4, H], f8, tag="wqt", name="wqt")
            wkt = cst.tile([128, 4, H], f8, tag="wkt", name="wkt")
            wvt = cst.tile([128, 4, H], f8, tag="wvt", name="wvt")
            w12t = cst.tile([128, 4, C], bf, tag="w12t", name="w12t")
            w2t = cst.tile([128, 2, C], f8, tag="w2t", name="w2t")
            scr = cst.tile([128, 1], f32, tag="scr", name="scr")

            xt8 = big.tile([128, 4, N], f8, tag="xt8", name="xt8")
            xtl8 = big.tile([128, 4, NL], f8, tag="xtl8", name="xtl8")
            xtlb = big.tile([128, 4, NL], bf, tag="xtlb", name="xtlb")
            qts = big.tile([128, 2, N], f8, tag="qts", name="qts")
            kts = big.tile([128, 2, NL], f8, tag="kts", name="kts")
            vts = [big.tile([128, 2, C], f8, tag=f"vts{t}", name=f"vts{t}")
                   for t in range(NPAIR)]
            vss = [big.tile([128, 2, C], f8, tag=f"vss{t}", name=f"vss{t}")
                   for t in range(NPAIR)]
            pts = [big.tile([128, 2, N], f8, tag=f"pts{t}", name=f"pts{t}")
                   for t in range(NPAIR)]
            colsq = big.tile([128, 4 * JT], f32, tag="colsq", name="colsq")
            csum = big.tile([128, JT], f32, tag="csum", name="csum")
            ginv = big.tile([128, JT], f32, tag="ginv", name="ginv")
            uts = big.tile([128, 2, NL], f8, tag="uts", name="uts")
            xyt = big.tile([128, 8, C], f32, tag="xyt", name="xyt")

            bqc = lambda hm: bpt[:, hm:hm + 1]
            bkc = lambda hm: bpt[:, 2 + hm:3 + hm]
            gsc = bpt[:, 4:5]
            bvt = bpt[:, 5:5 + C]
            b2t = bpt[:, 5 + C:5 + 2 * C]

            # ---- ACT table preload: Exp table load off the critical path
            nc.vector.memset(scr, 0.0)
            nc.scalar.activation(scr, scr, AF.Exp, scale=0.0)

            # ---- input DMAs ----
            nc.scalar.dma_start(out=wkt, in_=wk8[:])
            nc.scalar.dma_start(out=wqt, in_=wq8[:])
            nc.scalar.dma_start(out=bpt, in_=bpk[:])
            nc.gpsimd.dma_start(out=xtl8, in_=xTl8[:])
            nc.gpsimd.dma_start(out=wvt, in_=wv8[:])
            for c in range(NIC):
                sl = slice(c * 2048, (c + 1) * 2048)
                nc.sync.dma_start(out=xt8[:, :, sl], in_=xT8[:, :, sl])
            nc.sync.dma_start(out=xtlb, in_=xTlb[:])
            nc.sync.dma_start(out=w12t, in_=w12b[:])
            nc.sync.dma_start(out=w2t, in_=w2f[:])

            # ---- PE helpers ----
            def q_half(c, hm):
                """One hm-half of q^T[hq, i-chunk c]: 8 DR matmuls + a
                convert split across DVE/Pool at 1024 grain (so psum buffer
                rotation two allocs later never waits on a slow convert)."""
                qp = psum.tile([128, 2048], f32, tag="S", bufs=2, name="qp")
                for u in range(2):
                    for s in range(4):
                        nc.tensor.matmul(
                            qp[:, s * 512:(s + 1) * 512],
                            lhsT=wqt[:, 2 * u:2 * u + 2,
                                     hm * 128:(hm + 1) * 128],
                            rhs=xt8[:, 2 * u:2 * u + 2,
                                    c * 2048 + s * 512:c * 2048 + (s + 1) * 512],
                            start=(u == 0), stop=(u == 1), perf_mode=DR,
                        )
                for half, e in ((0, nc.vector), (1, nc.gpsimd)):
                    e.tensor_scalar_add(
                        qts[:, hm, c * 2048 + half * 1024:
                            c * 2048 + (half + 1) * 1024],
                        qp[:, half * 1024:(half + 1) * 1024], bqc(hm))

            def v_half(lo):
                """Half of the local v production (j-tiles lo..lo+3)."""
                vp = psum.tile([128, 2048], f32, tag="S", bufs=2, name="vp")
                for jt in range(lo, lo + 4):
                    for u in range(2):
                        nc.tensor.matmul(
                            vp[:, (jt - lo) * 256:(jt - lo + 1) * 256],
                            lhsT=xtl8[:, 2 * u:2 * u + 2,
                                      jt * 128:(jt + 1) * 128],
                            rhs=wvt[:, 2 * u:2 * u + 2, :],
                            start=(u == 0), stop=(u == 1), perf_mode=DR,
                        )
                for j2 in range(lo, lo + 4):
                    e = [nc.vector, nc.gpsimd][j2 % 2]
                    e.tensor_add(
                        vts[j2 // 2][:, j2 % 2, :],
                        vp[:, (j2 - lo) * 256:(j2 - lo + 1) * 256], bvt)

            # ---- head: k then q chunk 0 ----
            kp = psum.tile([128, 2048], f32, tag="S", bufs=2, name="kp")
            for hm in range(2):
                for jc in range(2):
                    for u in range(2):
                        nc.tensor.matmul(
                            kp[:, (hm * 2 + jc) * 512:(hm * 2 + jc + 1) * 512],
                            lhsT=wkt[:, 2 * u:2 * u + 2, hm * 128:(hm + 1) * 128],
                            rhs=xtl8[:, 2 * u:2 * u + 2, jc * 512:(jc + 1) * 512],
                            start=(u == 0), stop=(u == 1), perf_mode=DR,
                        )
            nc.gpsimd.tensor_scalar_add(kts[:, 0, :], kp[:, 0:1024], bkc(0))
            nc.vector.tensor_scalar_add(kts[:, 1, :], kp[:, 1024:2048], bkc(1))
            q_half(0, 0)
            q_half(0, 1)

            # per-(c,jt) PE work interleaved into the S stream: next q chunk
            # at jt 1/5, v production at jt 3/7 of chunk 0 -- each insertion
            # small enough that the exp pipeline never starves
            inserts = {}
            for c in range(NIC - 1):
                inserts[(c, 1)] = lambda cc=c: q_half(cc + 1, 0)
                inserts[(c, 5)] = lambda cc=c: q_half(cc + 1, 1)
            inserts[(0, 3)] = lambda: v_half(0)
            inserts[(0, 7)] = lambda: v_half(4)

            # ---- S + exp phase: i-chunk outer ----
            for c in range(NIC):
                for jt in range(JT):
                    sp = psum.tile([128, 2048], f32, tag="S", bufs=2,
                                   name="sp")
                    for s in range(4):
                        nc.tensor.matmul(
                            sp[:, s * 512:(s + 1) * 512],
                            lhsT=kts[:, :, jt * 128:(jt + 1) * 128],
                            rhs=qts[:, :, c * 2048 + s * 512:c * 2048 + (s + 1) * 512],
                            start=True, stop=True, perf_mode=DR,
                        )
                    pslice = pts[jt // 2][:, jt % 2, c * 2048:(c + 1) * 2048]
                    col = colsq[:, jt * 4 + c:jt * 4 + c + 1]
                    nc.scalar.activation(pslice, sp, AF.Exp,
                                         scale=EXPSCALE, accum_out=col)
                    if c == NIC - 1:
                        nc.vector.tensor_reduce(
                            csum[:, jt:jt + 1], colsq[:, jt * 4:(jt + 1) * 4],
                            mybir.AxisListType.X, mybir.AluOpType.add)
                        nc.vector.reciprocal(ginv[:, jt:jt + 1],
                                             csum[:, jt:jt + 1])
                        nc.vector.tensor_scalar_mul(
                            ginv[:, jt:jt + 1], ginv[:, jt:jt + 1], gsc)
                        if jt % 2 == 1:
                            t = jt // 2
                            for jm in range(2):
                                nc.vector.tensor_scalar_mul(
                                    vss[t][:, jm, :], vts[t][:, jm, :],
                                    ginv[:, 2 * t + jm:2 * t + jm + 1])
                    f = inserts.get((c, jt))
                    if f is not None:
                        f()

            # ---- PV phase: u''^T[c, i] partials, drained fp8 to pbuf ----
            drain_engs = [nc.gpsimd, nc.scalar]
            for g in range(4):
                stg = strm.tile([128, 2, 2, NL], f8, tag="stg", bufs=2,
                                name=f"stg{g}")  # [cp, r', ct, i']
                for ct in range(2):
                    pv = psum.tile([128, 2, NL], f32, tag="S", bufs=2,
                                   name="pv")    # [cp, r', i']
                    for sub in range(4):
                        ic = g * 4 + sub
                        for t in range(NPAIR):
                            nc.tensor.matmul(
                                pv[:, sub // 2, (sub % 2) * 512:(sub % 2 + 1) * 512],
                                lhsT=vss[t][:, :, ct * 128:(ct + 1) * 128],
                                rhs=pts[t][:, :, ic * 512:(ic + 1) * 512],
                                start=(t == 0), stop=(t == NPAIR - 1),
                                perf_mode=DR,
                            )
                    e = drain_engs[(g * 2 + ct) % 2]
                    if e is nc.scalar:
                        nc.scalar.copy(stg[:, :, ct, :], pv)
                    else:
                        e.tensor_scalar_add(stg[:, :, ct, :], pv, 0.0)
                    gb = 2 * 2 * 128 * NL
                    nc.sync.dma_start(
                        out=pbuf[g * gb:(g + 1) * gb].rearrange(
                            "(r p c i) -> p r c i", r=2, p=128, c=2)[:, :, ct, :],
                        in_=stg[:, :, ct, :],
                    )

            # ---- single collective: sum partials, scatter rows to owners ----
            nc.gpsimd.collective_compute(
                "ReduceScatter",
                mybir.AluOpType.add,
                replica_groups=RG,
                ins=[pbuf[:]],
                outs=[rsout[:]],
            )

            # ---- final: y = x@(a W1W2) + u''@(8 W2)/(8 VS) + b'' ----
            # x-term + PE warm-up dummies run DURING the ReduceScatter
            fx = psum.tile([128, 2048], f32, tag="S", bufs=2, name="fx")
            for it in range(8):
                for u in range(4):
                    nc.tensor.matmul(
                        fx[:, it * 256:(it + 1) * 256],
                        lhsT=xtlb[:, u, it * 128:(it + 1) * 128],
                        rhs=w12t[:, u, :],
                        start=(u == 0), stop=(u == 3),
                    )
            for it in range(8):
                e = [nc.vector, nc.gpsimd][it % 2]
                e.tensor_add(xyt[:, it, :],
                             fx[:, it * 256:(it + 1) * 256], b2t)
            nc.sync.dma_start(
                out=uts,
                in_=rsout[:].rearrange("(p c i) -> p c i", p=128, c=2),
            )
            # keep the PE p-state at full clock across the collective (slow
            # f32 matmuls into a scratch psum tile, no data deps on the RS);
            # the last few shrink so the overshoot past uts-arrival is small
            fdum = psum.tile([128, 2048], f32, tag="S", bufs=2, name="fdum")
            for d in range(NDUMMY):
                nc.tensor.matmul(
                    fdum[:, 0:512],
                    lhsT=bpt[:, 0:128],
                    rhs=bpt[:, 0:512],
                    start=True, stop=True,
                )
            for d in range(8):
                nc.tensor.matmul(
                    fdum[:, 0:128],
                    lhsT=bpt[:, 0:128],
                    rhs=bpt[:, 0:128],
                    start=True, stop=True,
                )
            fy = psum.tile([128, 2048], f32, tag="S", bufs=2, name="fy")
            for b in range(2):
                yst = strm.tile([128, 4, C], f32, tag="yst", bufs=2,
                                name=f"yst{b}")
                for qq in range(4):
                    it = b * 4 + qq
                    nc.tensor.matmul(
                        fy[:, it * 256:(it + 1) * 256],
                        lhsT=uts[:, :, it * 128:(it + 1) * 128],
                        rhs=w2t[:, :, :],
                        start=True, stop=True, perf_mode=DR,
                    )
                    e = [nc.vector, nc.gpsimd][it % 2]
                    e.scalar_tensor_tensor(
                        yst[:, qq, :],
                        fy[:, it * 256:(it + 1) * 256],
                        1.0 / (W2S * VS),
                        xyt[:, it, :],
                        op0=mybir.AluOpType.mult,
                        op1=mybir.AluOpType.add,
                    )
                nc.sync.dma_start(
                    out=y[b * 512:(b + 1) * 512, :].rearrange(
                        "(a p) c -> p a c", a=4, p=128),
                    in_=yst,
                )

    nc.finalize()
    return nc


def _get_nc():
    if "nc" not in _CACHED:
        _CACHED["nc"] = _build()
    return _CACHED["nc"]


def _prep_in_maps(x, W1, b1, Wq, bq, Wk, bk, Wv, bv, a, W2, b2):
    f32 = np.float32
    x = np.asarray(x, f32)
    W1 = np.asarray(W1, f32)
    b1 = np.asarray(b1, f32)
    av = f32(np.asarray(a, f32).reshape(-1)[0])

    def fold(Wx, bx):
        Wx, bx = np.asarray(Wx, f32), np.asarray(bx, f32)
        return W1 @ Wx, b1 @ Wx + bx

    Wqf, bqf = fold(Wq, bq)
    Wkf, bkf = fold(Wk, bk)
    Wvf, bvf = fold(Wv, bv)
    W2_ = np.asarray(W2, f32)
    W12 = av * (W1 @ W2_)
    b12 = av * (b1 @ W2_) + np.asarray(b2, f32)

    def pack(W, dtype, kt):
        return np.ascontiguousarray(
            W.reshape(kt, 128, W.shape[1]).transpose(1, 0, 2)).astype(dtype)

    def bcol(v2):
        return np.ascontiguousarray(v2.reshape(2, 128).T)

    bp = np.zeros((128, NBP), f32)
    bp[:, 0:2] = bcol(bqf * f32(WS))
    bp[:, 2:4] = bcol(bkf * f32(WS))
    bp[:, 4] = (f32(1.0) - av) * f32(VS)
    bp[:, 5:5 + C] = np.broadcast_to(bvf, (128, C))
    bp[:, 5 + C:5 + 2 * C] = np.broadcast_to(b12, (128, C))

    xT = np.ascontiguousarray(x.T.reshape(4, 128, N).transpose(1, 0, 2))
    xT8 = xT.astype(FP8)

    shared = {
        "xT8": xT8,
        "wq8": pack(Wqf * f32(WS), FP8, 4),
        "wk8": pack(Wkf * f32(WS), FP8, 4),
        "wv8": pack(Wvf, FP8, 4),
        "w12b": pack(W12, BF16, 4),
        "w2f": pack(W2_ * f32(W2S), FP8, 2),
        "bpk": bp,
    }
    maps = []
    for r in range(NCORES):
        sl = slice(r * NL, (r + 1) * NL)
        maps.append({
            **shared,
            "xTl8": np.ascontiguousarray(xT8[:, :, sl]),
            "xTlb": np.ascontiguousarray(xT[:, :, sl]).astype(BF16),
        })
    return maps


def kernel(**inputs) -> np.ndarray:
    from concourse.bass_utils import run_bass_kernel_spmd

    nc = _get_nc()
    in_maps = _prep_in_maps(**inputs)
    res = run_bass_kernel_spmd(nc, in_maps, list(range(NCORES)))
    return np.ascontiguousarray(
        np.concatenate(
            [res.results[r]["y"] for r in range(NCORES)], axis=0
        ).astype(np.float32)
    )


# revision 29
# speedup vs baseline: 2.2364x; 1.0141x over previous
"""Trainium2 Bass kernel for gated dense attention with dim=0 softmax.

Computation (reference):
    h = x @ W1 + b1
    q,k,v = h @ W{q,k,v} + b{q,k,v}
    w = (q @ k.T) / sqrt(256)
    attn = softmax(w, axis=0)          # normalizes over ROWS per column
    h2 = a*h + (1-a)*(attn @ v)
    out = h2 @ W2 + b2

Distribution strategy (chosen for the cost model's collective pricing:
every collective costs a flat ~15us + bytes/40GBps, AllReduce x1.875):

  Replicate x to all 8 cores as fp8 (4MB HBM->SBUF stream at 360GB/s is
  far cheaper than any AllGather at collective rates).  Core r owns
  COLUMN block J_r = [r*1024, (r+1)*1024) of the attention matrix: it
  computes q for ALL rows locally (x replicated), k,v only for its own
  rows, then S^r = k_r @ q_all^T -> [1024 j, 8192 i].  The dim=0
  softmax denominator sums over ALL i -- fully local in this layout (no
  stats AllReduce).  Each core forms the partial output
  u^r[i,:] = sum_{j in J_r} P[i,j] v'[j,:] for ALL i, and a single fp8
  ReduceScatter (256KB out, ~21.5us; reduction itself runs fp32) both
  sums the partials over cores and hands each core its own 1024 rows.

  Collectives: ONE ReduceScatter.  (Baseline: AG-k + AG-v + 4 stats
  AllReduces ~ 249us of serialized collective time.)

Schedule: the pacing engine is ScalarE doing the 8.4M exps/core.  exp
runs at [128,2048] grain from a 2-deep psum ping-pong, back-to-back at
1892ns; S matmuls (fp8 DoubleRow, ~1.8us/chunk) hide underneath, and
q-chunk production is interleaved INTO the S stream so the first exp
fires ~11us in.  Column sums run on the otherwise-idle Pool engine
(tensor_reduce over the just-written fp8 P tiles) except the last
i-chunk, which uses the exp's accum_out so the per-j scale is ready
the moment its final exp retires.  During the ReduceScatter the PE
computes the residual x@(a*W1W2) term and then chews dummy f32 matmuls
to stay at full clock for the post-collective projection.

Weight folding (host, O(weights) only): q = x @ (W1@Wq) + (b1@Wq + bq),
same for k,v; y = x @ (a*W1@W2) + u'' @ (8*W2) / (8*VS) + (a*b1@W2+b2)
with u'' = VS*(1-a)*u carried through the fp8 ReduceScatter.
"""

import numpy as np
import ml_dtypes

BF16 = ml_dtypes.bfloat16
FP8 = ml_dtypes.float8_e4m3fn

N, D, H, C = 8192, 512, 256, 256
NCORES = 8
NL = N // NCORES          # 1024 rows/columns per core
JT = NL // 128            # 8 local j-tiles
NPAIR = JT // 2           # 4 DoubleRow j-pairs
NIC = N // 2048           # 4 i-chunks at the 2048-wide exp grain
WS = 16.0                 # fp8 range scale on folded Wq/Wk
VS = 4096.0               # range shim on v' and u'' (u'' stays in fp8 range)
W2S = 8.0                 # fp8 range scale on W2
EXPSCALE = 1.0 / (WS * WS * 16.0)   # restores exp(q.k/sqrt(256))
NBP = 5 + 2 * C           # packed bias columns
NDUMMY = 25               # f32 warm-up matmuls spanning the ReduceScatter

_CACHED = {}


def _build():
    import concourse.mybir as mybir
    from concourse import bacc
    from concourse.tile import TileContext

    dt = mybir.dt
    AF = mybir.ActivationFunctionType
    DR = mybir.MatmulPerfMode.DoubleRow
    f32, bf, f8 = dt.float32, dt.bfloat16, dt.float8e4
    RG = [list(range(NCORES))]

    nc = bacc.Bacc(None, target_bir_lowering=False, num_devices=NCORES)

    # ---------------- I/O (per core) ----------------
    xT8 = nc.declare_dram_parameter("xT8", [128, 4, N], f8, isOutput=False)
    xTl8 = nc.declare_dram_parameter("xTl8", [128, 4, NL], f8, isOutput=False)
    xTlb = nc.declare_dram_parameter("xTlb", [128, 4, NL], bf, isOutput=False)
    wqT8 = nc.declare_dram_parameter("wqT8", [128, 2, D], f8, isOutput=False)
    bq8 = nc.declare_dram_parameter("bq8", [128, 2, 1], f8, isOutput=False)
    wk8 = nc.declare_dram_parameter("wk8", [128, 4, H], f8, isOutput=False)
    wv8 = nc.declare_dram_parameter("wv8", [128, 4, H], f8, isOutput=False)
    w12b = nc.declare_dram_parameter("w12b", [128, 4, C], bf, isOutput=False)
    w2f = nc.declare_dram_parameter("w2f", [128, 2, C], f8, isOutput=False)
    bpk = nc.declare_dram_parameter("bpk", [128, NBP], f32, isOutput=False)
    y = nc.declare_dram_parameter("y", [NL, C], f32, isOutput=True)

    # partial outputs, laid out [rank][cp, ct, i'] so the ReduceScatter
    # shard for rank r is u''^T for its own rows, lhsT-ready
    pbuf = nc.dram_tensor("pbuf", [NCORES * 2 * 128 * NL], f8)
    rsout = nc.dram_tensor("rsout", [2 * 128 * NL], f8)

    with TileContext(nc) as tc:
        with (
            tc.tile_pool(name="cst", bufs=1) as cst,
            tc.tile_pool(name="big", bufs=1) as big,
            tc.tile_pool(name="strm", bufs=1) as strm,
            tc.tile_pool(name="psum", bufs=1, space="PSUM") as psum,
        ):
            # ---- SBUF residents ----
            bpt = cst.tile([128, NBP], f32, tag="bpt", name="bpt")
            wqtt = cst.tile([128, 2, D], f8, tag="wqtt", name="wqtt")
            bqt8 = cst.tile([128, 2, 1], f8, tag="bqt8", name="bqt8")
            wkt = cst.tile([128, 4, H], f8, tag="wkt", name="wkt")
            wvt = cst.tile([128, 4, H], f8, tag="wvt", name="wvt")
            w12t = cst.tile([128, 4, C], bf, tag="w12t", name="w12t")
            w2t = cst.tile([128, 2, C], f8, tag="w2t", name="w2t")
            scr = cst.tile([128, 1], f32, tag="scr", name="scr")

            xt8 = big.tile([128, 4, N], f8, tag="xt8", name="xt8")
            xtl8 = big.tile([128, 4, NL], f8, tag="xtl8", name="xtl8")
            xtlb = big.tile([128, 4, NL], bf, tag="xtlb", name="xtlb")
            mts = big.tile([128, 4, NL], f8, tag="mts", name="mts")
            bqm = big.tile([128, JT], f32, tag="bqm", name="bqm")
            kts = big.tile([128, 2, NL], f8, tag="kts", name="kts")
            vts = [big.tile([128, 2, C], f8, tag=f"vts{t}", name=f"vts{t}")
                   for t in range(NPAIR)]
            vss = [big.tile([128, 2, C], f8, tag=f"vss{t}", name=f"vss{t}")
                   for t in range(NPAIR)]
            pts = [big.tile([128, 2, N], f8, tag=f"pts{t}", name=f"pts{t}")
                   for t in range(NPAIR)]
            colsq = big.tile([128, 4 * JT], f32, tag="colsq", name="colsq")
            csum = big.tile([128, JT], f32, tag="csum", name="csum")
            ginv = big.tile([128, JT], f32, tag="ginv", name="ginv")
            uts = big.tile([128, 2, NL], f8, tag="uts", name="uts")
            xyt = big.tile([128, 8, C], f32, tag="xyt", name="xyt")

            bqc = lambda hm: bpt[:, hm:hm + 1]
            bkc = lambda hm: bpt[:, 2 + hm:3 + hm]
            gsc = bpt[:, 4:5]
            bvt = bpt[:, 5:5 + C]
            b2t = bpt[:, 5 + C:5 + 2 * C]

            # ---- ACT table preload: Exp table load off the critical path
            nc.vector.memset(scr, 0.0)
            nc.scalar.activation(scr, scr, AF.Exp, scale=0.0)

            # ---- input DMAs ----
            nc.scalar.dma_start(out=wkt, in_=wk8[:])
            nc.scalar.dma_start(out=bpt, in_=bpk[:])
            nc.scalar.dma_start(out=wqtt, in_=wqT8[:])
            nc.scalar.dma_start(out=bqt8, in_=bq8[:])
            nc.gpsimd.dma_start(out=xtl8, in_=xTl8[:])
            nc.gpsimd.dma_start(out=wvt, in_=wv8[:])
            for c in range(NIC):
                sl = slice(c * 2048, (c + 1) * 2048)
                nc.sync.dma_start(out=xt8[:, :, sl], in_=xT8[:, :, sl])
            nc.sync.dma_start(out=xtlb, in_=xTlb[:])
            nc.sync.dma_start(out=w12t, in_=w12b[:])
            nc.sync.dma_start(out=w2t, in_=w2f[:])

            # ---- head: k, v, then M^T = Wq'' @ k^T so the S contraction
            # runs against the replicated x directly (q never materializes;
            # its bias enters via the exp's per-partition bias). The S/exp
            # stream then needs NO interleaved producer work at all.
            kp = psum.tile([128, 2048], f32, tag="S", bufs=2, name="kp")
            for hm in range(2):
                for jc in range(2):
                    for u in range(2):
                        nc.tensor.matmul(
                            kp[:, (hm * 2 + jc) * 512:(hm * 2 + jc + 1) * 512],
                            lhsT=wkt[:, 2 * u:2 * u + 2, hm * 128:(hm + 1) * 128],
                            rhs=xtl8[:, 2 * u:2 * u + 2, jc * 512:(jc + 1) * 512],
                            start=(u == 0), stop=(u == 1), perf_mode=DR,
                        )
            nc.gpsimd.tensor_scalar_add(kts[:, 0, :], kp[:, 0:1024], bkc(0))
            nc.vector.tensor_scalar_add(kts[:, 1, :], kp[:, 1024:2048], bkc(1))

            # v[j_local, c] while the k conversion drains
            vp = psum.tile([128, 2048], f32, tag="S", bufs=2, name="vp")
            for jt in range(JT):
                for u in range(2):
                    nc.tensor.matmul(
                        vp[:, jt * 256:(jt + 1) * 256],
                        lhsT=xtl8[:, 2 * u:2 * u + 2, jt * 128:(jt + 1) * 128],
                        rhs=wvt[:, 2 * u:2 * u + 2, :],
                        start=(u == 0), stop=(u == 1), perf_mode=DR,
                    )
            for j2 in range(JT):
                e = [nc.vector, nc.gpsimd][j2 % 2]
                e.tensor_add(vts[j2 // 2][:, j2 % 2, :],
                             vp[:, j2 * 256:(j2 + 1) * 256], bvt)

            # M^T[hin, j] tiles (fp8) + the per-j bias column k.bq
            mp = [psum.tile([128, 2048], f32, tag="S", bufs=2, name=f"mp{h}")
                  for h in range(2)]
            for hk in range(4):
                for jc in range(2):
                    nc.tensor.matmul(
                        mp[hk // 2][:, (hk % 2) * 1024 + jc * 512:
                                    (hk % 2) * 1024 + (jc + 1) * 512],
                        lhsT=wqtt[:, :, hk * 128:(hk + 1) * 128],
                        rhs=kts[:, :, jc * 512:(jc + 1) * 512],
                        start=True, stop=True, perf_mode=DR,
                    )
            bqp = psum.tile([128, 2048], f32, tag="S", bufs=2, name="bqp")
            for jt in range(JT):
                nc.tensor.matmul(
                    bqp[:, jt:jt + 1],
                    lhsT=kts[:, :, jt * 128:(jt + 1) * 128],
                    rhs=bqt8,
                    start=True, stop=True, perf_mode=DR,
                )
            for h in range(2):
                for e2 in range(2):
                    eng = [nc.vector, nc.gpsimd][(2 * h + e2) % 2]
                    eng.tensor_scalar_add(
                        mts[:, 2 * h + e2, :],
                        mp[h][:, e2 * 1024:(e2 + 1) * 1024], 0.0)
            nc.vector.tensor_scalar_mul(bqm, bqp[:, 0:JT], 1.0 / 4096.0)

            # ---- S + exp phase: i-chunk outer, completely uniform ----
            for c in range(NIC):
                for jt in range(JT):
                    sp = psum.tile([128, 2048], f32, tag="S", bufs=2,
                                   name="sp")
                    for s in range(4):
                        for u in range(2):
                            nc.tensor.matmul(
                                sp[:, s * 512:(s + 1) * 512],
                                lhsT=mts[:, 2 * u:2 * u + 2,
                                         jt * 128:(jt + 1) * 128],
                                rhs=xt8[:, 2 * u:2 * u + 2,
                                        c * 2048 + s * 512:c * 2048 + (s + 1) * 512],
                                start=(u == 0), stop=(u == 1), perf_mode=DR,
                            )
                    pslice = pts[jt // 2][:, jt % 2, c * 2048:(c + 1) * 2048]
                    col = colsq[:, jt * 4 + c:jt * 4 + c + 1]
                    nc.scalar.activation(pslice, sp, AF.Exp,
                                         scale=EXPSCALE,
                                         bias=bqm[:, jt:jt + 1],
                                         accum_out=col)
                    if c == NIC - 1:
                        nc.vector.tensor_reduce(
                            csum[:, jt:jt + 1], colsq[:, jt * 4:(jt + 1) * 4],
                            mybir.AxisListType.X, mybir.AluOpType.add)
                        nc.vector.reciprocal(ginv[:, jt:jt + 1],
                                             csum[:, jt:jt + 1])
                        nc.vector.tensor_scalar_mul(
                            ginv[:, jt:jt + 1], ginv[:, jt:jt + 1], gsc)
                        if jt % 2 == 1:
                            t = jt // 2
                            for jm in range(2):
                                nc.vector.tensor_scalar_mul(
                                    vss[t][:, jm, :], vts[t][:, jm, :],
                                    ginv[:, 2 * t + jm:2 * t + jm + 1])

            # ---- PV phase: u''^T[c, i] partials, drained fp8 to pbuf ----
            drain_engs = [nc.gpsimd, nc.scalar]
            for g in range(4):
                stg = strm.tile([128, 2, 2, NL], f8, tag="stg", bufs=2,
                                name=f"stg{g}")  # [cp, r', ct, i']
                for ct in range(2):
                    pv = psum.tile([128, 2, NL], f32, tag="S", bufs=2,
                                   name="pv")    # [cp, r', i']
                    for sub in range(4):
                        ic = g * 4 + sub
                        for t in range(NPAIR):
                            nc.tensor.matmul(
                                pv[:, sub // 2, (sub % 2) * 512:(sub % 2 + 1) * 512],
                                lhsT=vss[t][:, :, ct * 128:(ct + 1) * 128],
                                rhs=pts[t][:, :, ic * 512:(ic + 1) * 512],
                                start=(t == 0), stop=(t == NPAIR - 1),
                                perf_mode=DR,
                            )
                    e = drain_engs[(g * 2 + ct) % 2]
                    if e is nc.scalar:
                        nc.scalar.copy(stg[:, :, ct, :], pv)
                    else:
                        e.tensor_scalar_add(stg[:, :, ct, :], pv, 0.0)
                    gb = 2 * 2 * 128 * NL
                    nc.sync.dma_start(
                        out=pbuf[g * gb:(g + 1) * gb].rearrange(
                            "(r p c i) -> p r c i", r=2, p=128, c=2)[:, :, ct, :],
                        in_=stg[:, :, ct, :],
                    )

            # ---- single collective: sum partials, scatter rows to owners ----
            nc.gpsimd.collective_compute(
                "ReduceScatter",
                mybir.AluOpType.add,
                replica_groups=RG,
                ins=[pbuf[:]],
                outs=[rsout[:]],
            )

            # ---- final: y = x@(a W1W2) + u''@(8 W2)/(8 VS) + b'' ----
            # x-term + PE warm-up dummies run DURING the ReduceScatter
            fx = psum.tile([128, 2048], f32, tag="S", bufs=2, name="fx")
            for it in range(8):
                for u in range(4):
                    nc.tensor.matmul(
                        fx[:, it * 256:(it + 1) * 256],
                        lhsT=xtlb[:, u, it * 128:(it + 1) * 128],
                        rhs=w12t[:, u, :],
                        start=(u == 0), stop=(u == 3),
                    )
            for it in range(8):
                e = [nc.vector, nc.gpsimd][it % 2]
                e.tensor_add(xyt[:, it, :],
                             fx[:, it * 256:(it + 1) * 256], b2t)
            nc.sync.dma_start(
                out=uts,
                in_=rsout[:].rearrange("(p c i) -> p c i", p=128, c=2),
            )
            # keep the PE p-state at full clock across the collective (slow
            # f32 matmuls into a scratch psum tile, no data deps on the RS);
            # the last few shrink so the overshoot past uts-arrival is small
            fdum = psum.tile([128, 2048], f32, tag="S", bufs=2, name="fdum")
            for d in range(NDUMMY):
                nc.tensor.matmul(
                    fdum[:, 0:512],
                    lhsT=bpt[:, 0:128],
                    rhs=bpt[:, 0:512],
                    start=True, stop=True,
                )
            for d in range(8):
                nc.tensor.matmul(
                    fdum[:, 0:128],
                    lhsT=bpt[:, 0:128],
                    rhs=bpt[:, 0:128],
                    start=True, stop=True,
                )
            fy = psum.tile([128, 2048], f32, tag="S", bufs=2, name="fy")
            for b in range(2):
                yst = strm.tile([128, 4, C], f32, tag="yst", bufs=2,
                                name=f"yst{b}")
                for qq in range(4):
                    it = b * 4 + qq
                    nc.tensor.matmul(
                        fy[:, it * 256:(it + 1) * 256],
                        lhsT=uts[:, :, it * 128:(it + 1) * 128],
                        rhs=w2t[:, :, :],
                        start=True, stop=True, perf_mode=DR,
                    )
                    e = [nc.vector, nc.gpsimd][it % 2]
                    e.scalar_tensor_tensor(
                        yst[:, qq, :],
                        fy[:, it * 256:(it + 1) * 256],
                        1.0 / (W2S * VS),
                        xyt[:, it, :],
                        op0=mybir.AluOpType.mult,
                        op1=mybir.AluOpType.add,
                    )
                nc.sync.dma_start(
                    out=y[b * 512:(b + 1) * 512, :].rearrange(
                        "(a p) c -> p a c", a=4, p=128),
                    in_=yst,
                )

    nc.finalize()
    return nc


def _get_nc():
    if "nc" not in _CACHED:
        _CACHED["nc"] = _build()
    return _CACHED["nc"]


def _prep_in_maps(x, W1, b1, Wq, bq, Wk, bk, Wv, bv, a, W2, b2):
    f32 = np.float32
    x = np.asarray(x, f32)
    W1 = np.asarray(W1, f32)
    b1 = np.asarray(b1, f32)
    av = f32(np.asarray(a, f32).reshape(-1)[0])

    def fold(Wx, bx):
        Wx, bx = np.asarray(Wx, f32), np.asarray(bx, f32)
        return W1 @ Wx, b1 @ Wx + bx

    Wqf, bqf = fold(Wq, bq)
    Wkf, bkf = fold(Wk, bk)
    Wvf, bvf = fold(Wv, bv)
    W2_ = np.asarray(W2, f32)
    W12 = av * (W1 @ W2_)
    b12 = av * (b1 @ W2_) + np.asarray(b2, f32)

    def pack(W, dtype, kt):
        return np.ascontiguousarray(
            W.reshape(kt, 128, W.shape[1]).transpose(1, 0, 2)).astype(dtype)

    def bcol(v2):
        return np.ascontiguousarray(v2.reshape(2, 128).T)

    bp = np.zeros((128, NBP), f32)
    bp[:, 0:2] = bcol(bqf * f32(WS))
    bp[:, 2:4] = bcol(bkf * f32(WS))
    bp[:, 4] = (f32(1.0) - av) * f32(VS)
    bp[:, 5:5 + C] = np.broadcast_to(bvf, (128, C))
    bp[:, 5 + C:5 + 2 * C] = np.broadcast_to(b12, (128, C))

    xT = np.ascontiguousarray(x.T.reshape(4, 128, N).transpose(1, 0, 2))
    xT8 = xT.astype(FP8)

    shared = {
        "xT8": xT8,
        "wqT8": pack(np.ascontiguousarray((Wqf * f32(WS)).T), FP8, 2),
        "bq8": np.ascontiguousarray(
            (bqf * f32(WS)).reshape(2, 128).T)[:, :, None].astype(FP8),
        "wk8": pack(Wkf * f32(WS), FP8, 4),
        "wv8": pack(Wvf, FP8, 4),
        "w12b": pack(W12, BF16, 4),
        "w2f": pack(W2_ * f32(W2S), FP8, 2),
        "bpk": bp,
    }
    maps = []
    for r in range(NCORES):
        sl = slice(r * NL, (r + 1) * NL)
        maps.append({
            **shared,
            "xTl8": np.ascontiguousarray(xT8[:, :, sl]),
            "xTlb": np.ascontiguousarray(xT[:, :, sl]).astype(BF16),
        })
    return maps


def kernel(**inputs) -> np.ndarray:
    from concourse.bass_utils import run_bass_kernel_spmd

    nc = _get_nc()
    in_maps = _prep_in_maps(**inputs)
    res = run_bass_kernel_spmd(nc, in_maps, list(range(NCORES)))
    return np.ascontiguousarray(
        np.concatenate(
            [res.results[r]["y"] for r in range(NCORES)], axis=0
        ).astype(np.float32)
    )


# revision 37
# speedup vs baseline: 2.2998x; 1.0284x over previous
"""Trainium2 Bass kernel for gated dense attention with dim=0 softmax.

Computation (reference):
    h = x @ W1 + b1
    q,k,v = h @ W{q,k,v} + b{q,k,v}
    w = (q @ k.T) / sqrt(256)
    attn = softmax(w, axis=0)          # normalizes over ROWS per column
    h2 = a*h + (1-a)*(attn @ v)
    out = h2 @ W2 + b2

Distribution strategy (chosen for the cost model's collective pricing:
every collective costs a flat ~15us + bytes/40GBps, AllReduce x1.875):

  Replicate x to all 8 cores as fp8 (4MB HBM->SBUF stream at 360GB/s is
  far cheaper than any AllGather at collective rates).  Core r owns
  COLUMN block J_r = [r*1024, (r+1)*1024) of the attention matrix: it
  computes q for ALL rows locally (x replicated), k,v only for its own
  rows, then S^r = k_r @ q_all^T -> [1024 j, 8192 i].  The dim=0
  softmax denominator sums over ALL i -- fully local in this layout (no
  stats AllReduce).  Each core forms the partial output
  u^r[i,:] = sum_{j in J_r} P[i,j] v'[j,:] for ALL i, and a single fp8
  ReduceScatter (256KB out, ~21.5us; reduction itself runs fp32) both
  sums the partials over cores and hands each core its own 1024 rows.

  Collectives: ONE ReduceScatter.  (Baseline: AG-k + AG-v + 4 stats
  AllReduces ~ 249us of serialized collective time.)

Schedule: the pacing engine is ScalarE doing the 8.4M exps/core.  exp
runs at [128,2048] grain from a 2-deep psum ping-pong, back-to-back at
1892ns; S matmuls (fp8 DoubleRow, ~1.8us/chunk) hide underneath, and
q-chunk production is interleaved INTO the S stream so the first exp
fires ~11us in.  Column sums run on the otherwise-idle Pool engine
(tensor_reduce over the just-written fp8 P tiles) except the last
i-chunk, which uses the exp's accum_out so the per-j scale is ready
the moment its final exp retires.  During the ReduceScatter the PE
computes the residual x@(a*W1W2) term and then chews dummy f32 matmuls
to stay at full clock for the post-collective projection.

Weight folding (host, O(weights) only): q = x @ (W1@Wq) + (b1@Wq + bq),
same for k,v; y = x @ (a*W1@W2) + u'' @ (8*W2) / (8*VS) + (a*b1@W2+b2)
with u'' = VS*(1-a)*u carried through the fp8 ReduceScatter.
"""

import numpy as np
import ml_dtypes

BF16 = ml_dtypes.bfloat16
FP8 = ml_dtypes.float8_e4m3fn

N, D, H, C = 8192, 512, 256, 256
NCORES = 8
NL = N // NCORES          # 1024 rows/columns per core
JT = NL // 128            # 8 local j-tiles
NPAIR = JT // 2           # 4 DoubleRow j-pairs
NIC = N // 2048           # 4 i-chunks at the 2048-wide exp grain
WS = 16.0                 # fp8 range scale on folded Wq/Wk
VS = 4096.0               # range shim on v' and u'' (u'' stays in fp8 range)
W2S = 8.0                 # fp8 range scale on W2
EXPSCALE = 1.0 / (WS * WS * 16.0)   # restores exp(q.k/sqrt(256))
NBP = 10 + 2 * C          # packed bias columns (gsc, bv, b12, gb x4, cb)
NDUMMY = 25               # f32 warm-up matmuls spanning the ReduceScatter

_CACHED = {}


def _build():
    import concourse.mybir as mybir
    from concourse import bacc
    from concourse.tile import TileContext

    dt = mybir.dt
    AF = mybir.ActivationFunctionType
    DR = mybir.MatmulPerfMode.DoubleRow
    f32, bf, f8 = dt.float32, dt.bfloat16, dt.float8e4
    RG = [list(range(NCORES))]

    nc = bacc.Bacc(None, target_bir_lowering=False, num_devices=NCORES)

    # ---------------- I/O (per core) ----------------
    xT8 = nc.declare_dram_parameter("xT8", [128, 4, N], f8, isOutput=False)
    xTl8 = nc.declare_dram_parameter("xTl8", [128, 4, NL], f8, isOutput=False)
    xTlb = nc.declare_dram_parameter("xTlb", [128, 4, NL], bf, isOutput=False)
    g8 = nc.declare_dram_parameter("g8", [128, 4, D], f8, isOutput=False)
    wb8 = nc.declare_dram_parameter("wb8", [128, 4, 1], f8, isOutput=False)
    wv8 = nc.declare_dram_parameter("wv8", [128, 4, H], f8, isOutput=False)
    w12b = nc.declare_dram_parameter("w12b", [128, 4, C], bf, isOutput=False)
    w2f = nc.declare_dram_parameter("w2f", [128, 2, C], f8, isOutput=False)
    bpk = nc.declare_dram_parameter("bpk", [128, NBP], f32, isOutput=False)
    y = nc.declare_dram_parameter("y", [NL, C], f32, isOutput=True)

    # partial outputs, laid out [rank][cp, ct, i'] so the ReduceScatter
    # shard for rank r is u''^T for its own rows, lhsT-ready
    pbuf = nc.dram_tensor("pbuf", [NCORES * 2 * 128 * NL], f8)
    rsout = nc.dram_tensor("rsout", [2 * 128 * NL], f8)

    with TileContext(nc) as tc:
        with (
            tc.tile_pool(name="cst", bufs=1) as cst,
            tc.tile_pool(name="big", bufs=1) as big,
            tc.tile_pool(name="strm", bufs=1) as strm,
            tc.tile_pool(name="psum", bufs=1, space="PSUM") as psum,
        ):
            # ---- SBUF residents ----
            bpt = cst.tile([128, NBP], f32, tag="bpt", name="bpt")
            g8t = cst.tile([128, 4, D], f8, tag="g8t", name="g8t")
            wb8t = cst.tile([128, 4, 1], f8, tag="wb8t", name="wb8t")
            wvt = cst.tile([128, 4, H], f8, tag="wvt", name="wvt")
            w12t = cst.tile([128, 4, C], bf, tag="w12t", name="w12t")
            w2t = cst.tile([128, 2, C], f8, tag="w2t", name="w2t")
            scr = cst.tile([128, 1], f32, tag="scr", name="scr")

            xt8 = big.tile([128, 4, N], f8, tag="xt8", name="xt8")
            xtl8 = big.tile([128, 4, NL], f8, tag="xtl8", name="xtl8")
            xtlb = big.tile([128, 4, NL], bf, tag="xtlb", name="xtlb")
            mts = big.tile([128, 4, NL], f8, tag="mts", name="mts")
            bqm = big.tile([128, JT], f32, tag="bqm", name="bqm")
            vts = [big.tile([128, 2, C], f8, tag=f"vts{t}", name=f"vts{t}")
                   for t in range(NPAIR)]
            vss = [big.tile([128, 2, C], f8, tag=f"vss{t}", name=f"vss{t}")
                   for t in range(NPAIR)]
            pts = [big.tile([128, 2, N], f8, tag=f"pts{t}", name=f"pts{t}")
                   for t in range(NPAIR)]
            colsq = big.tile([128, 4 * JT], f32, tag="colsq", name="colsq")
            csum = big.tile([128, JT], f32, tag="csum", name="csum")
            ginv = big.tile([128, JT], f32, tag="ginv", name="ginv")
            uts = big.tile([128, 2, NL], f8, tag="uts", name="uts")
            xyt = big.tile([128, 8, C], f32, tag="xyt", name="xyt")

            gsc = bpt[:, 4:5]
            bvt = bpt[:, 5:5 + C]
            b2t = bpt[:, 5 + C:5 + 2 * C]
            gbc = lambda t: bpt[:, 5 + 2 * C + t:6 + 2 * C + t]
            cbc = bpt[:, 9 + 2 * C:10 + 2 * C]

            # ---- ACT table preload: Exp table load off the critical path
            nc.vector.memset(scr, 0.0)
            nc.scalar.activation(scr, scr, AF.Exp, scale=0.0)

            # ---- input DMAs ----
            nc.scalar.dma_start(out=g8t, in_=g8[:])
            nc.scalar.dma_start(out=bpt, in_=bpk[:])
            nc.scalar.dma_start(out=wb8t, in_=wb8[:])
            nc.gpsimd.dma_start(out=xtl8, in_=xTl8[:])
            nc.gpsimd.dma_start(out=wvt, in_=wv8[:])
            for c in range(NIC):
                sl = slice(c * 2048, (c + 1) * 2048)
                nc.sync.dma_start(out=xt8[:, :, sl], in_=xT8[:, :, sl])
            nc.sync.dma_start(out=xtlb, in_=xTlb[:])
            nc.sync.dma_start(out=w12t, in_=w12b[:])
            nc.sync.dma_start(out=w2t, in_=w2f[:])

            # ---- head: M^T = G @ x_l^T + gb where G = 256*(Wq' Wk'^T) is
            # host-folded, so S contracts M directly against the replicated
            # x (neither q nor k ever materializes; the q-bias row enters
            # via the exp's per-partition bias, itself a folded matvec).
            mp = [psum.tile([128, 2048], f32, tag="S", bufs=2, name=f"mp{h}")
                  for h in range(2)]
            for hk in range(4):
                for jc in range(2):
                    for u in range(2):
                        nc.tensor.matmul(
                            mp[hk // 2][:, (hk % 2) * 1024 + jc * 512:
                                        (hk % 2) * 1024 + (jc + 1) * 512],
                            lhsT=g8t[:, 2 * u:2 * u + 2, hk * 128:(hk + 1) * 128],
                            rhs=xtl8[:, 2 * u:2 * u + 2, jc * 512:(jc + 1) * 512],
                            start=(u == 0), stop=(u == 1), perf_mode=DR,
                        )
            # exp-bias matvec: bias_j = x_l[j].(Wk' bq)/16 + bk.bq/16
            bqp = psum.tile([128, 2048], f32, tag="S", bufs=2, name="bqp")
            for jt in range(JT):
                for u in range(2):
                    nc.tensor.matmul(
                        bqp[:, jt:jt + 1],
                        lhsT=xtl8[:, 2 * u:2 * u + 2, jt * 128:(jt + 1) * 128],
                        rhs=wb8t[:, 2 * u:2 * u + 2, :],
                        start=(u == 0), stop=(u == 1), perf_mode=DR,
                    )
            # v[j_local, c]
            vp = psum.tile([128, 2048], f32, tag="S", bufs=2, name="vp")
            for jt in range(JT):
                for u in range(2):
                    nc.tensor.matmul(
                        vp[:, jt * 256:(jt + 1) * 256],
                        lhsT=xtl8[:, 2 * u:2 * u + 2, jt * 128:(jt + 1) * 128],
                        rhs=wvt[:, 2 * u:2 * u + 2, :],
                        start=(u == 0), stop=(u == 1), perf_mode=DR,
                    )
            for h in range(2):
                for e2 in range(2):
                    eng = [nc.vector, nc.gpsimd][(2 * h + e2) % 2]
                    eng.tensor_scalar_add(
                        mts[:, 2 * h + e2, :],
                        mp[h][:, e2 * 1024:(e2 + 1) * 1024], gbc(2 * h + e2))
            nc.vector.tensor_scalar(
                bqm, bqp[:, 0:JT], 1.0 / 1024.0, cbc,
                op0=mybir.AluOpType.mult, op1=mybir.AluOpType.add)
            for j2 in range(JT):
                e = [nc.vector, nc.gpsimd][j2 % 2]
                e.tensor_add(vts[j2 // 2][:, j2 % 2, :],
                             vp[:, j2 * 256:(j2 + 1) * 256], bvt)

            # ---- S + exp phase: i-chunk outer, completely uniform ----
            for c in range(NIC):
                for jt in range(JT):
                    sp = psum.tile([128, 2048], f32, tag="S", bufs=2,
                                   name="sp")
                    for s in range(4):
                        for u in range(2):
                            nc.tensor.matmul(
                                sp[:, s * 512:(s + 1) * 512],
                                lhsT=mts[:, 2 * u:2 * u + 2,
                                         jt * 128:(jt + 1) * 128],
                                rhs=xt8[:, 2 * u:2 * u + 2,
                                        c * 2048 + s * 512:c * 2048 + (s + 1) * 512],
                                start=(u == 0), stop=(u == 1), perf_mode=DR,
                            )
                    pslice = pts[jt // 2][:, jt % 2, c * 2048:(c + 1) * 2048]
                    col = colsq[:, jt * 4 + c:jt * 4 + c + 1]
                    nc.scalar.activation(pslice, sp, AF.Exp,
                                         scale=EXPSCALE,
                                         bias=bqm[:, jt:jt + 1],
                                         accum_out=col)
                    if c == NIC - 1:
                        nc.vector.tensor_reduce(
                            csum[:, jt:jt + 1], colsq[:, jt * 4:(jt + 1) * 4],
                            mybir.AxisListType.X, mybir.AluOpType.add)
                        nc.vector.reciprocal(ginv[:, jt:jt + 1],
                                             csum[:, jt:jt + 1])
                        nc.vector.tensor_scalar_mul(
                            ginv[:, jt:jt + 1], ginv[:, jt:jt + 1], gsc)
                        if jt % 2 == 1:
                            t = jt // 2
                            for jm in range(2):
                                nc.vector.tensor_scalar_mul(
                                    vss[t][:, jm, :], vts[t][:, jm, :],
                                    ginv[:, 2 * t + jm:2 * t + jm + 1])

            # ---- PV phase: u''^T[c, i] partials, drained fp8 to pbuf ----
            drain_engs = [nc.gpsimd, nc.scalar]
            for g in range(4):
                stg = strm.tile([128, 2, 2, NL], f8, tag="stg", bufs=2,
                                name=f"stg{g}")  # [cp, r', ct, i']
                for ct in range(2):
                    pv = psum.tile([128, 2, NL], f32, tag="S", bufs=2,
                                   name="pv")    # [cp, r', i']
                    for sub in range(4):
                        ic = g * 4 + sub
                        for t in range(NPAIR):
                            nc.tensor.matmul(
                                pv[:, sub // 2, (sub % 2) * 512:(sub % 2 + 1) * 512],
                                lhsT=vss[t][:, :, ct * 128:(ct + 1) * 128],
                                rhs=pts[t][:, :, ic * 512:(ic + 1) * 512],
                                start=(t == 0), stop=(t == NPAIR - 1),
                                perf_mode=DR,
                            )
                    e = drain_engs[(g * 2 + ct) % 2]
                    if e is nc.scalar:
                        nc.scalar.copy(stg[:, :, ct, :], pv)
                    else:
                        e.tensor_scalar_add(stg[:, :, ct, :], pv, 0.0)
                    gb = 2 * 2 * 128 * NL
                    nc.sync.dma_start(
                        out=pbuf[g * gb:(g + 1) * gb].rearrange(
                            "(r p c i) -> p r c i", r=2, p=128, c=2)[:, :, ct, :],
                        in_=stg[:, :, ct, :],
                    )

            # ---- single collective: sum partials, scatter rows to owners ----
            nc.gpsimd.collective_compute(
                "ReduceScatter",
                mybir.AluOpType.add,
                replica_groups=RG,
                ins=[pbuf[:]],
                outs=[rsout[:]],
            )

            # ---- final: y = x@(a W1W2) + u''@(8 W2)/(8 VS) + b'' ----
            # x-term + PE warm-up dummies run DURING the ReduceScatter
            fx = psum.tile([128, 2048], f32, tag="S", bufs=2, name="fx")
            for it in range(8):
                for u in range(4):
                    nc.tensor.matmul(
                        fx[:, it * 256:(it + 1) * 256],
                        lhsT=xtlb[:, u, it * 128:(it + 1) * 128],
                        rhs=w12t[:, u, :],
                        start=(u == 0), stop=(u == 3),
                    )
            for it in range(8):
                e = [nc.vector, nc.gpsimd][it % 2]
                e.tensor_add(xyt[:, it, :],
                             fx[:, it * 256:(it + 1) * 256], b2t)
            nc.sync.dma_start(
                out=uts,
                in_=rsout[:].rearrange("(p c i) -> p c i", p=128, c=2),
            )
            # keep the PE p-state at full clock across the collective (slow
            # f32 matmuls into a scratch psum tile, no data deps on the RS);
            # the last few shrink so the overshoot past uts-arrival is small
            fdum = psum.tile([128, 2048], f32, tag="S", bufs=2, name="fdum")
            for d in range(NDUMMY):
                nc.tensor.matmul(
                    fdum[:, 0:512],
                    lhsT=bpt[:, 0:128],
                    rhs=bpt[:, 0:512],
                    start=True, stop=True,
                )
            for d in range(8):
                nc.tensor.matmul(
                    fdum[:, 0:128],
                    lhsT=bpt[:, 0:128],
                    rhs=bpt[:, 0:128],
                    start=True, stop=True,
                )
            # one psum BANK per it-slice (512-f32 stride): the yst add that
            # reads slice it must not share a bank with the it+1 matmul or
            # the bank-granular WAR check serializes the projection stream
            fya = psum.tile([128, 2048], f32, tag="S", bufs=2, name="fya")
            fyb = psum.tile([128, 2048], f32, tag="S", bufs=2, name="fyb")
            for b in range(2):
                yst = strm.tile([128, 4, C], f32, tag="yst", bufs=2,
                                name=f"yst{b}")
                for qq in range(4):
                    it = b * 4 + qq
                    fslice = [fya, fyb][it // 4][:, (it % 4) * 512:
                                                 (it % 4) * 512 + 256]
                    nc.tensor.matmul(
                        fslice,
                        lhsT=uts[:, :, it * 128:(it + 1) * 128],
                        rhs=w2t[:, :, :],
                        start=True, stop=True, perf_mode=DR,
                    )
                    e = [nc.vector, nc.gpsimd][it % 2]
                    e.scalar_tensor_tensor(
                        yst[:, qq, :],
                        fslice,
                        1.0 / (W2S * VS),
                        xyt[:, it, :],
                        op0=mybir.AluOpType.mult,
                        op1=mybir.AluOpType.add,
                    )
                nc.sync.dma_start(
                    out=y[b * 512:(b + 1) * 512, :].rearrange(
                        "(a p) c -> p a c", a=4, p=128),
                    in_=yst,
                )

    nc.finalize()
    return nc


def _get_nc():
    if "nc" not in _CACHED:
        _CACHED["nc"] = _build()
    return _CACHED["nc"]


def _prep_in_maps(x, W1, b1, Wq, bq, Wk, bk, Wv, bv, a, W2, b2):
    f32 = np.float32
    x = np.asarray(x, f32)
    W1 = np.asarray(W1, f32)
    b1 = np.asarray(b1, f32)
    av = f32(np.asarray(a, f32).reshape(-1)[0])

    def fold(Wx, bx):
        Wx, bx = np.asarray(Wx, f32), np.asarray(bx, f32)
        return W1 @ Wx, b1 @ Wx + bx

    Wqf, bqf = fold(Wq, bq)
    Wkf, bkf = fold(Wk, bk)
    Wvf, bvf = fold(Wv, bv)
    W2_ = np.asarray(W2, f32)
    W12 = av * (W1 @ W2_)
    b12 = av * (b1 @ W2_) + np.asarray(b2, f32)

    def pack(W, dtype, kt):
        return np.ascontiguousarray(
            W.reshape(kt, 128, W.shape[1]).transpose(1, 0, 2)).astype(dtype)

    def bcol(v2):
        return np.ascontiguousarray(v2.reshape(2, 128).T)

    # Gram-folded attention-score factors (see module docstring):
    #   S_psum = (256 Wq' Wk'^T x_l^T)^T-contracted-with-x, exp bias =
    #   x_l.(64 Wk' bq)/1024 + (bk.bq)/16 per local row j
    G = f32(256.0) * (Wqf @ Wkf.T)              # [512 hin, 512 din]
    gb = f32(256.0) * (Wqf @ bkf)               # [512 hin]
    wb = f32(64.0) * (Wkf @ bqf)                # [512 din]
    cb = f32(np.dot(bkf, bqf) / 16.0)           # scalar

    bp = np.zeros((128, NBP), f32)
    bp[:, 4] = (f32(1.0) - av) * f32(VS)
    bp[:, 5:5 + C] = np.broadcast_to(bvf, (128, C))
    bp[:, 5 + C:5 + 2 * C] = np.broadcast_to(b12, (128, C))
    bp[:, 5 + 2 * C:9 + 2 * C] = gb.reshape(4, 128).T
    bp[:, 9 + 2 * C] = cb

    xT = np.ascontiguousarray(x.T.reshape(4, 128, N).transpose(1, 0, 2))
    xT8 = xT.astype(FP8)

    shared = {
        "xT8": xT8,
        "g8": pack(np.ascontiguousarray(G.T), FP8, 4),
        "wb8": np.ascontiguousarray(
            wb.reshape(4, 128).T)[:, :, None].astype(FP8),
        "wv8": pack(Wvf, FP8, 4),
        "w12b": pack(W12, BF16, 4),
        "w2f": pack(W2_ * f32(W2S), FP8, 2),
        "bpk": bp,
    }
    maps = []
    for r in range(NCORES):
        sl = slice(r * NL, (r + 1) * NL)
        maps.append({
            **shared,
            "xTl8": np.ascontiguousarray(xT8[:, :, sl]),
            "xTlb": np.ascontiguousarray(xT[:, :, sl]).astype(BF16),
        })
    return maps


def kernel(**inputs) -> np.ndarray:
    from concourse.bass_utils import run_bass_kernel_spmd

    nc = _get_nc()
    in_maps = _prep_in_maps(**inputs)
    res = run_bass_kernel_spmd(nc, in_maps, list(range(NCORES)))
    return np.ascontiguousarray(
        np.concatenate(
            [res.results[r]["y"] for r in range(NCORES)], axis=0
        ).astype(np.float32)
    )


# revision 40
# speedup vs baseline: 2.3443x; 1.0193x over previous
"""Trainium2 Bass kernel for gated dense attention with dim=0 softmax.

Computation (reference):
    h = x @ W1 + b1
    q,k,v = h @ W{q,k,v} + b{q,k,v}
    w = (q @ k.T) / sqrt(256)
    attn = softmax(w, axis=0)          # normalizes over ROWS per column
    h2 = a*h + (1-a)*(attn @ v)
    out = h2 @ W2 + b2

Distribution strategy (chosen for the cost model's collective pricing:
every collective costs a flat ~15us + bytes/40GBps, AllReduce x1.875):

  Replicate x to all 8 cores as fp8 (4MB HBM->SBUF stream at 360GB/s is
  far cheaper than any AllGather at collective rates).  Core r owns
  COLUMN block J_r = [r*1024, (r+1)*1024) of the attention matrix: it
  computes q for ALL rows locally (x replicated), k,v only for its own
  rows, then S^r = k_r @ q_all^T -> [1024 j, 8192 i].  The dim=0
  softmax denominator sums over ALL i -- fully local in this layout (no
  stats AllReduce).  Each core forms the partial output
  u^r[i,:] = sum_{j in J_r} P[i,j] v'[j,:] for ALL i, and a single fp8
  ReduceScatter (256KB out, ~21.5us; reduction itself runs fp32) both
  sums the partials over cores and hands each core its own 1024 rows.

  Collectives: ONE ReduceScatter.  (Baseline: AG-k + AG-v + 4 stats
  AllReduces ~ 249us of serialized collective time.)

Schedule: the pacing engine is ScalarE doing the 8.4M exps/core.  exp
runs at [128,2048] grain from a 2-deep psum ping-pong, back-to-back at
1892ns; S matmuls (fp8 DoubleRow, ~1.8us/chunk) hide underneath, and
q-chunk production is interleaved INTO the S stream so the first exp
fires ~11us in.  Column sums run on the otherwise-idle Pool engine
(tensor_reduce over the just-written fp8 P tiles) except the last
i-chunk, which uses the exp's accum_out so the per-j scale is ready
the moment its final exp retires.  During the ReduceScatter the PE
computes the residual x@(a*W1W2) term and then chews dummy f32 matmuls
to stay at full clock for the post-collective projection.

Weight folding (host, O(weights) only): q = x @ (W1@Wq) + (b1@Wq + bq),
same for k,v; y = x @ (a*W1@W2) + u'' @ (8*W2) / (8*VS) + (a*b1@W2+b2)
with u'' = VS*(1-a)*u carried through the fp8 ReduceScatter.
"""

import numpy as np
import ml_dtypes

BF16 = ml_dtypes.bfloat16
FP8 = ml_dtypes.float8_e4m3fn

N, D, H, C = 8192, 512, 256, 256
NCORES = 8
NL = N // NCORES          # 1024 rows/columns per core
JT = NL // 128            # 8 local j-tiles
NPAIR = JT // 2           # 4 DoubleRow j-pairs
NIC = N // 2048           # 4 i-chunks at the 2048-wide exp grain
WS = 16.0                 # fp8 range scale on folded Wq/Wk
VS = 4096.0               # range shim on v' and u'' (u'' stays in fp8 range)
W2S = 8.0                 # fp8 range scale on W2
EXPSCALE = 1.0 / (WS * WS * 16.0)   # restores exp(q.k/sqrt(256))
NBP = 10 + 2 * C          # packed bias columns (gsc, bv, b12, gb x4, cb)
NDUMMY = 28               # f32 warm-up matmuls spanning the ReduceScatter

_CACHED = {}


def _build():
    import concourse.mybir as mybir
    from concourse import bacc
    from concourse.tile import TileContext

    dt = mybir.dt
    AF = mybir.ActivationFunctionType
    DR = mybir.MatmulPerfMode.DoubleRow
    f32, bf, f8 = dt.float32, dt.bfloat16, dt.float8e4
    RG = [list(range(NCORES))]

    nc = bacc.Bacc(None, target_bir_lowering=False, num_devices=NCORES)

    # ---------------- I/O (per core) ----------------
    xT8 = nc.declare_dram_parameter("xT8", [128, 4, N], f8, isOutput=False)
    xTl8 = nc.declare_dram_parameter("xTl8", [128, 4, NL], f8, isOutput=False)
    xTlb = nc.declare_dram_parameter("xTlb", [128, 4, NL], bf, isOutput=False)
    g8 = nc.declare_dram_parameter("g8", [128, 4, D], f8, isOutput=False)
    wb8 = nc.declare_dram_parameter("wb8", [128, 4, 1], f8, isOutput=False)
    wv8 = nc.declare_dram_parameter("wv8", [128, 4, H], f8, isOutput=False)
    w12b = nc.declare_dram_parameter("w12b", [128, 4, C], bf, isOutput=False)
    w2f = nc.declare_dram_parameter("w2f", [128, 2, C], f8, isOutput=False)
    bpk = nc.declare_dram_parameter("bpk", [128, NBP], f32, isOutput=False)
    y = nc.declare_dram_parameter("y", [NL, C], f32, isOutput=True)

    # partial outputs, laid out [rank][cp, ct, i'] so the ReduceScatter
    # shard for rank r is u''^T for its own rows, lhsT-ready
    pbuf = nc.dram_tensor("pbuf", [NCORES * 2 * 128 * NL], f8)
    rsout = nc.dram_tensor("rsout", [2 * 128 * NL], f8)

    with TileContext(nc) as tc:
        with (
            tc.tile_pool(name="cst", bufs=1) as cst,
            tc.tile_pool(name="big", bufs=1) as big,
            tc.tile_pool(name="strm", bufs=1) as strm,
            tc.tile_pool(name="psum", bufs=1, space="PSUM") as psum,
        ):
            # ---- SBUF residents ----
            bpt = cst.tile([128, NBP], f32, tag="bpt", name="bpt")
            g8t = cst.tile([128, 4, D], f8, tag="g8t", name="g8t")
            wb8t = cst.tile([128, 4, 1], f8, tag="wb8t", name="wb8t")
            wvt = cst.tile([128, 4, H], f8, tag="wvt", name="wvt")
            w12t = cst.tile([128, 4, C], bf, tag="w12t", name="w12t")
            w2t = cst.tile([128, 2, C], f8, tag="w2t", name="w2t")
            scr = cst.tile([128, 1], f32, tag="scr", name="scr")

            xt8 = big.tile([128, 4, N], f8, tag="xt8", name="xt8")
            xtl8 = big.tile([128, 4, NL], f8, tag="xtl8", name="xtl8")
            xtlb = big.tile([128, 4, NL], bf, tag="xtlb", name="xtlb")
            mts = big.tile([128, 4, NL], f8, tag="mts", name="mts")
            bqm = big.tile([128, JT], f32, tag="bqm", name="bqm")
            vts = [big.tile([128, 2, C], f8, tag=f"vts{t}", name=f"vts{t}")
                   for t in range(NPAIR)]
            vss = [big.tile([128, 2, C], f8, tag=f"vss{t}", name=f"vss{t}")
                   for t in range(NPAIR)]
            pts = [big.tile([128, 2, N], f8, tag=f"pts{t}", name=f"pts{t}")
                   for t in range(NPAIR)]
            colsq = big.tile([128, 4 * JT], f32, tag="colsq", name="colsq")
            csum = big.tile([128, JT], f32, tag="csum", name="csum")
            ginv = big.tile([128, JT], f32, tag="ginv", name="ginv")
            uts = big.tile([128, 2, NL], f8, tag="uts", name="uts")
            xyt = big.tile([128, 8, C], f32, tag="xyt", name="xyt")

            gsc = bpt[:, 4:5]
            bvt = bpt[:, 5:5 + C]
            b2t = bpt[:, 5 + C:5 + 2 * C]
            gbc = lambda t: bpt[:, 5 + 2 * C + t:6 + 2 * C + t]
            cbc = bpt[:, 9 + 2 * C:10 + 2 * C]

            # ---- ACT table preload: Exp table load off the critical path
            nc.vector.memset(scr, 0.0)
            nc.scalar.activation(scr, scr, AF.Exp, scale=0.0)

            # ---- input DMAs ----
            nc.scalar.dma_start(out=g8t, in_=g8[:])
            nc.scalar.dma_start(out=bpt, in_=bpk[:])
            nc.scalar.dma_start(out=wb8t, in_=wb8[:])
            nc.gpsimd.dma_start(out=xtl8, in_=xTl8[:])
            nc.gpsimd.dma_start(out=wvt, in_=wv8[:])
            for c in range(NIC):
                sl = slice(c * 2048, (c + 1) * 2048)
                nc.sync.dma_start(out=xt8[:, :, sl], in_=xT8[:, :, sl])
            nc.sync.dma_start(out=xtlb, in_=xTlb[:])
            nc.sync.dma_start(out=w12t, in_=w12b[:])
            nc.sync.dma_start(out=w2t, in_=w2f[:])

            # ---- head: M^T = G @ x_l^T + gb where G = 256*(Wq' Wk'^T) is
            # host-folded, so S contracts M directly against the replicated
            # x (neither q nor k ever materializes; the q-bias row enters
            # via the exp's per-partition bias, itself a folded matvec).
            # exp-bias matvec first: bias_j = x_l[j].(Wk' bq)/16 + bk.bq/16
            bqp = psum.tile([128, 2048], f32, tag="S", bufs=2, name="bqp")
            for jt in range(JT):
                for u in range(2):
                    nc.tensor.matmul(
                        bqp[:, jt:jt + 1],
                        lhsT=xtl8[:, 2 * u:2 * u + 2, jt * 128:(jt + 1) * 128],
                        rhs=wb8t[:, 2 * u:2 * u + 2, :],
                        start=(u == 0), stop=(u == 1), perf_mode=DR,
                    )
            mp = [psum.tile([128, 2048], f32, tag="S", bufs=2, name=f"mp{h}")
                  for h in range(2)]
            for hk in range(4):
                for jc in range(2):
                    for u in range(2):
                        nc.tensor.matmul(
                            mp[hk // 2][:, (hk % 2) * 1024 + jc * 512:
                                        (hk % 2) * 1024 + (jc + 1) * 512],
                            lhsT=g8t[:, 2 * u:2 * u + 2, hk * 128:(hk + 1) * 128],
                            rhs=xtl8[:, 2 * u:2 * u + 2, jc * 512:(jc + 1) * 512],
                            start=(u == 0), stop=(u == 1), perf_mode=DR,
                        )
            nc.vector.tensor_scalar(
                bqm, bqp[:, 0:JT], 1.0 / 1024.0, cbc,
                op0=mybir.AluOpType.mult, op1=mybir.AluOpType.add)
            for h in range(2):
                for e2 in range(2):
                    eng = [nc.vector, nc.gpsimd][(2 * h + e2) % 2]
                    eng.tensor_scalar_add(
                        mts[:, 2 * h + e2, :],
                        mp[h][:, e2 * 1024:(e2 + 1) * 1024], gbc(2 * h + e2))

            def v_mms():
                # v[j_local, c]: slotted after the first S chunk so it does
                # not delay the first exp (PE slack under the exp pipeline)
                vp = psum.tile([128, 2048], f32, tag="S", bufs=2, name="vp")
                for jt in range(JT):
                    for u in range(2):
                        nc.tensor.matmul(
                            vp[:, jt * 256:(jt + 1) * 256],
                            lhsT=xtl8[:, 2 * u:2 * u + 2, jt * 128:(jt + 1) * 128],
                            rhs=wvt[:, 2 * u:2 * u + 2, :],
                            start=(u == 0), stop=(u == 1), perf_mode=DR,
                        )
                for j2 in range(JT):
                    nc.gpsimd.tensor_add(vts[j2 // 2][:, j2 % 2, :],
                                         vp[:, j2 * 256:(j2 + 1) * 256], bvt)

            # ---- S + exp phase: i-chunk outer, completely uniform.
            # Column sums for the first three i-chunks run as DVE reduces
            # over the just-written fp8 P tiles (saves the 187ns accumulator
            # drain on the pacing ScalarE); the last i-chunk keeps accum_out
            # so the per-j scale is ready the moment its exp retires.
            for c in range(NIC):
                for jt in range(JT):
                    sp = psum.tile([128, 2048], f32, tag="S", bufs=2,
                                   name="sp")
                    for s in range(4):
                        for u in range(2):
                            nc.tensor.matmul(
                                sp[:, s * 512:(s + 1) * 512],
                                lhsT=mts[:, 2 * u:2 * u + 2,
                                         jt * 128:(jt + 1) * 128],
                                rhs=xt8[:, 2 * u:2 * u + 2,
                                        c * 2048 + s * 512:c * 2048 + (s + 1) * 512],
                                start=(u == 0), stop=(u == 1), perf_mode=DR,
                            )
                    pslice = pts[jt // 2][:, jt % 2, c * 2048:(c + 1) * 2048]
                    col = colsq[:, jt * 4 + c:jt * 4 + c + 1]
                    if c < NIC - 1:
                        nc.scalar.activation(pslice, sp, AF.Exp,
                                             scale=EXPSCALE,
                                             bias=bqm[:, jt:jt + 1])
                        nc.vector.tensor_reduce(
                            col, pslice, mybir.AxisListType.X,
                            mybir.AluOpType.add)
                    else:
                        nc.scalar.activation(pslice, sp, AF.Exp,
                                             scale=EXPSCALE,
                                             bias=bqm[:, jt:jt + 1],
                                             accum_out=col)
                    if c == 0 and jt == 0:
                        v_mms()
                    if c == NIC - 1:
                        nc.vector.tensor_reduce(
                            csum[:, jt:jt + 1], colsq[:, jt * 4:(jt + 1) * 4],
                            mybir.AxisListType.X, mybir.AluOpType.add)
                        nc.vector.reciprocal(ginv[:, jt:jt + 1],
                                             csum[:, jt:jt + 1])
                        nc.vector.tensor_scalar_mul(
                            ginv[:, jt:jt + 1], ginv[:, jt:jt + 1], gsc)
                        if jt % 2 == 1:
                            t = jt // 2
                            for jm in range(2):
                                nc.vector.tensor_scalar_mul(
                                    vss[t][:, jm, :], vts[t][:, jm, :],
                                    ginv[:, 2 * t + jm:2 * t + jm + 1])

            # ---- PV phase: u''^T[c, i] partials, drained fp8 to pbuf ----
            drain_engs = [nc.gpsimd, nc.scalar]
            for g in range(4):
                stg = strm.tile([128, 2, 2, NL], f8, tag="stg", bufs=2,
                                name=f"stg{g}")  # [cp, r', ct, i']
                for ct in range(2):
                    pv = psum.tile([128, 2, NL], f32, tag="S", bufs=2,
                                   name="pv")    # [cp, r', i']
                    for sub in range(4):
                        ic = g * 4 + sub
                        for t in range(NPAIR):
                            nc.tensor.matmul(
                                pv[:, sub // 2, (sub % 2) * 512:(sub % 2 + 1) * 512],
                                lhsT=vss[t][:, :, ct * 128:(ct + 1) * 128],
                                rhs=pts[t][:, :, ic * 512:(ic + 1) * 512],
                                start=(t == 0), stop=(t == NPAIR - 1),
                                perf_mode=DR,
                            )
                    e = drain_engs[(g * 2 + ct) % 2]
                    if e is nc.scalar:
                        nc.scalar.copy(stg[:, :, ct, :], pv)
                    else:
                        e.tensor_scalar_add(stg[:, :, ct, :], pv, 0.0)
                    gb = 2 * 2 * 128 * NL
                    nc.sync.dma_start(
                        out=pbuf[g * gb:(g + 1) * gb].rearrange(
                            "(r p c i) -> p r c i", r=2, p=128, c=2)[:, :, ct, :],
                        in_=stg[:, :, ct, :],
                    )

            # ---- single collective: sum partials, scatter rows to owners ----
            nc.gpsimd.collective_compute(
                "ReduceScatter",
                mybir.AluOpType.add,
                replica_groups=RG,
                ins=[pbuf[:]],
                outs=[rsout[:]],
            )

            # ---- final: y = x@(a W1W2) + u''@(8 W2)/(8 VS) + b'' ----
            # x-term + PE warm-up dummies run DURING the ReduceScatter
            fx = psum.tile([128, 2048], f32, tag="S", bufs=2, name="fx")
            for it in range(8):
                for u in range(4):
                    nc.tensor.matmul(
                        fx[:, it * 256:(it + 1) * 256],
                        lhsT=xtlb[:, u, it * 128:(it + 1) * 128],
                        rhs=w12t[:, u, :],
                        start=(u == 0), stop=(u == 3),
                    )
            for it in range(8):
                e = [nc.vector, nc.gpsimd][it % 2]
                e.tensor_add(xyt[:, it, :],
                             fx[:, it * 256:(it + 1) * 256], b2t)
            nc.sync.dma_start(
                out=uts,
                in_=rsout[:].rearrange("(p c i) -> p c i", p=128, c=2),
            )
            # keep the PE p-state at full clock across the collective (slow
            # f32 matmuls into a scratch psum tile, no data deps on the RS);
            # the last few shrink so the overshoot past uts-arrival is small
            fdum = psum.tile([128, 2048], f32, tag="S", bufs=2, name="fdum")
            for d in range(NDUMMY):
                nc.tensor.matmul(
                    fdum[:, 0:512],
                    lhsT=bpt[:, 0:128],
                    rhs=bpt[:, 0:512],
                    start=True, stop=True,
                )
            for d in range(8):
                nc.tensor.matmul(
                    fdum[:, 0:128],
                    lhsT=bpt[:, 0:128],
                    rhs=bpt[:, 0:128],
                    start=True, stop=True,
                )
            # one psum BANK per it-slice (512-f32 stride): the yst add that
            # reads slice it must not share a bank with the it+1 matmul or
            # the bank-granular WAR check serializes the projection stream
            fya = psum.tile([128, 2048], f32, tag="S", bufs=2, name="fya")
            fyb = psum.tile([128, 2048], f32, tag="S", bufs=2, name="fyb")
            for b in range(4):
                yst = strm.tile([128, 2, C], f32, tag="yst", bufs=2,
                                name=f"yst{b}")
                for qq in range(2):
                    it = b * 2 + qq
                    fslice = [fya, fyb][it // 4][:, (it % 4) * 512:
                                                 (it % 4) * 512 + 256]
                    nc.tensor.matmul(
                        fslice,
                        lhsT=uts[:, :, it * 128:(it + 1) * 128],
                        rhs=w2t[:, :, :],
                        start=True, stop=True, perf_mode=DR,
                    )
                    e = [nc.vector, nc.gpsimd][it % 2]
                    e.scalar_tensor_tensor(
                        yst[:, qq, :],
                        fslice,
                        1.0 / (W2S * VS),
                        xyt[:, it, :],
                        op0=mybir.AluOpType.mult,
                        op1=mybir.AluOpType.add,
                    )
                nc.sync.dma_start(
                    out=y[b * 256:(b + 1) * 256, :].rearrange(
                        "(a p) c -> p a c", a=2, p=128),
                    in_=yst,
                )

    nc.finalize()
    return nc


def _get_nc():
    if "nc" not in _CACHED:
        _CACHED["nc"] = _build()
    return _CACHED["nc"]


def _prep_in_maps(x, W1, b1, Wq, bq, Wk, bk, Wv, bv, a, W2, b2):
    f32 = np.float32
    x = np.asarray(x, f32)
    W1 = np.asarray(W1, f32)
    b1 = np.asarray(b1, f32)
    av = f32(np.asarray(a, f32).reshape(-1)[0])

    def fold(Wx, bx):
        Wx, bx = np.asarray(Wx, f32), np.asarray(bx, f32)
        return W1 @ Wx, b1 @ Wx + bx

    Wqf, bqf = fold(Wq, bq)
    Wkf, bkf = fold(Wk, bk)
    Wvf, bvf = fold(Wv, bv)
    W2_ = np.asarray(W2, f32)
    W12 = av * (W1 @ W2_)
    b12 = av * (b1 @ W2_) + np.asarray(b2, f32)

    def pack(W, dtype, kt):
        return np.ascontiguousarray(
            W.reshape(kt, 128, W.shape[1]).transpose(1, 0, 2)).astype(dtype)

    def bcol(v2):
        return np.ascontiguousarray(v2.reshape(2, 128).T)

    # Gram-folded attention-score factors (see module docstring):
    #   S_psum = (256 Wq' Wk'^T x_l^T)^T-contracted-with-x, exp bias =
    #   x_l.(64 Wk' bq)/1024 + (bk.bq)/16 per local row j
    G = f32(256.0) * (Wqf @ Wkf.T)              # [512 hin, 512 din]
    gb = f32(256.0) * (Wqf @ bkf)               # [512 hin]
    wb = f32(64.0) * (Wkf @ bqf)                # [512 din]
    cb = f32(np.dot(bkf, bqf) / 16.0)           # scalar

    bp = np.zeros((128, NBP), f32)
    bp[:, 4] = (f32(1.0) - av) * f32(VS)
    bp[:, 5:5 + C] = np.broadcast_to(bvf, (128, C))
    bp[:, 5 + C:5 + 2 * C] = np.broadcast_to(b12, (128, C))
    bp[:, 5 + 2 * C:9 + 2 * C] = gb.reshape(4, 128).T
    bp[:, 9 + 2 * C] = cb

    xT = np.ascontiguousarray(x.T.reshape(4, 128, N).transpose(1, 0, 2))
    xT8 = xT.astype(FP8)

    shared = {
        "xT8": xT8,
        "g8": pack(np.ascontiguousarray(G.T), FP8, 4),
        "wb8": np.ascontiguousarray(
            wb.reshape(4, 128).T)[:, :, None].astype(FP8),
        "wv8": pack(Wvf, FP8, 4),
        "w12b": pack(W12, BF16, 4),
        "w2f": pack(W2_ * f32(W2S), FP8, 2),
        "bpk": bp,
    }
    maps = []
    for r in range(NCORES):
        sl = slice(r * NL, (r + 1) * NL)
        maps.append({
            **shared,
            "xTl8": np.ascontiguousarray(xT8[:, :, sl]),
            "xTlb": np.ascontiguousarray(xT[:, :, sl]).astype(BF16),
        })
    return maps


def kernel(**inputs) -> np.ndarray:
    from concourse.bass_utils import run_bass_kernel_spmd

    nc = _get_nc()
    in_maps = _prep_in_maps(**inputs)
    res = run_bass_kernel_spmd(nc, in_maps, list(range(NCORES)))
    return np.ascontiguousarray(
        np.concatenate(
            [res.results[r]["y"] for r in range(NCORES)], axis=0
        ).astype(np.float32)
    )
